# revision 75
# baseline (speedup 1.0000x reference)
"""nn_CausalWanSelfAttention Trainium2 kernel (8-core SPMD, single launch).

Entry point: kernel(**inputs) -> np.ndarray [1, 6240, 1536] float32.

Strategy (v2 — bf16 datapath, packed kv, software-pipelined attention):
  - Phase A, token-sharded (780 tokens/core): q/k/v projections as bf16
    matmuls (f32 PSUM accumulation), rmsnorm via bf16 square + ones-matmul
    partition reduction. 3D-RoPE on pair-de-interleaved channels: the
    partition-half swap runs on the DMA engines (SBUF->SBUF), then
    dst = raw*[fr;fr] + swap(raw)*[-fi;fi] as same-partition bf16 ops
    (2x DVE mode); the final add fuses the phi token-reduction via
    scalar_tensor_tensor(accum_out=...). The per-token rmsnorm scale
    folds into the rope tables once per projection.
  - One AllGather ships (k^T, v) in bf16 (780 tokens per block, no pad);
    one small AllReduce combines the routing means phi_q/phi_k.
  - Top-2-of-4 chunk routing on device; per-head chunk indices drive
    dynamically-addressed DMA gathers of the selected K/V blocks.
  - Phase B, query-sharded, emitted as ONE flat software-pipelined stream
    over all (head, kv-tile) steps: the logits matmul for step i+1 is
    emitted before step i's PV so the in-order PE queue never head-blocks
    on the exp semaphore, and each head's denominator reduce / reciprocal
    / o^T normalization are deferred several steps into the next head.
    24 full 128-row kv tiles plus ONE packed tail tile assembled from the
    4 selected blocks' 12 leftover rows (25 tile-ops per engine, not 28).
    exp on the scalar engine (the pacing engine, ~87% busy); softmax
    denominator via a bf16 pairwise tree on the vector engine; PV
    accumulation in PSUM; the row-parallel output projection reads o^T
    (bf16) from SBUF.
"""

from contextlib import ExitStack

import numpy as np

import concourse.bacc as bacc
import concourse.mybir as mybir
import concourse.tile as tile

F32R = mybir.dt.float32r
F32 = mybir.dt.float32
BF16 = mybir.dt.bfloat16
KV_BYTES_PER_ELEM = 2    # K/V ship over the AllGather in bf16

N_CORES = 8
S, D, NH, HD, C = 6240, 1536, 12, 128, 64
NT = D // 128          # 12 channel tiles
TOK = S // N_CORES     # 780 tokens per core
NBLK = N_CORES         # 8 kv blocks (one per core)
NCH = 4                # routing chunks
NKT = 6                # full 128-row kv tiles per block (768 rows)
KTAIL = TOK - NKT * 128   # 12 leftover kv rows per block
NTAIL = 4 * KTAIL         # 48 packed tail rows (4 selected blocks)
EPS = 1e-6
SM_SCALE = 1.0 / float(np.sqrt(HD))
KV_ELEMS = NT * 128 * TOK  # bf16 elements per (k or v) per core
MTAIL = TOK - 6 * 128  # 12 (token tail within a core block)


def build_kernel(n_cores=N_CORES, solo=False, phases="full"):
    nc = bacc.Bacc("TRN2", target_bir_lowering=False, debug=False,
                   num_devices=n_cores)

    xT = nc.dram_tensor("xT", [NT, 128, TOK], BF16, kind="ExternalInput")
    wqT = nc.dram_tensor("wqT", [NT, 128, D], BF16, kind="ExternalInput")
    wkT = nc.dram_tensor("wkT", [NT, 128, D], BF16, kind="ExternalInput")
    wvT = nc.dram_tensor("wvT", [NT, 128, D], BF16, kind="ExternalInput")
    woT = nc.dram_tensor("woT", [NT, 128, D], BF16, kind="ExternalInput")
    f11 = nc.dram_tensor("f11", [128, TOK], BF16, kind="ExternalInput")
    f2n = nc.dram_tensor("f2n", [128, TOK], BF16, kind="ExternalInput")
    chmask = nc.dram_tensor("chmask", [128, NCH], F32, kind="ExternalInput")

    out = nc.dram_tensor("out", [TOK, D], F32, kind="ExternalOutput")

    # collective buffers
    ag_in = nc.dram_tensor("ag_in", [2, KV_ELEMS], BF16)
    ag_out = nc.dram_tensor("ag_out", [NBLK, 2, KV_ELEMS], BF16,
                            addr_space="Shared")
    phi_in = nc.dram_tensor("phi_in", [128, NT, 1 + NCH], F32)
    phi_out = nc.dram_tensor("phi_out", [128, NT, 1 + NCH], F32,
                             addr_space="Shared")
    rec_dram = nc.dram_tensor("rec_dram", [NH, TOK], BF16)

    k_in_view = ag_in.ap()[0].rearrange("(t p n) -> t p n", p=128, n=TOK)
    v_in_view = ag_in.ap()[1].rearrange("(k d) -> k d", d=D)

    ones_col_t = nc.inline_tensor(np.ones((128, 1), np.float32), name="ones_col")
    ones_row_t = nc.inline_tensor(np.ones((1, 128), np.float32), name="ones_row")

    with tile.TileContext(nc) as tc, ExitStack() as top:
        consts = top.enter_context(tc.tile_pool(name="consts", bufs=1))
        ones_col = consts.tile([128, 1], F32R)
        nc.sync.dma_start(out=ones_col, in_=ones_col_t.ap().bitcast(F32R))
        ones_row = consts.tile([1, 128], F32R)
        nc.sync.dma_start(out=ones_row, in_=ones_row_t.ap().bitcast(F32R))
        ones_col_bf = consts.tile([128, 1], BF16)
        nc.vector.memset(ones_col_bf, 1.0)
        f11_sb = consts.tile([128, TOK], BF16)
        nc.sync.dma_start(out=f11_sb, in_=f11[:, :])
        f2n_sb = consts.tile([128, TOK], BF16)
        nc.sync.dma_start(out=f2n_sb, in_=f2n[:, :])
        cm_sb = consts.tile([128, NCH], F32)
        nc.sync.dma_start(out=cm_sb, in_=chmask[:, :])
        eps_sb = consts.tile([1, 1], F32)
        nc.vector.memset(eps_sb, EPS)
        # preload the Exp activation table so the first attention exp
        # doesn't pay the 1.3us LoadActFuncSet at the phase boundary
        warm = consts.tile([1, 1], F32)
        nc.scalar.activation(out=warm, in_=eps_sb,
                             func=mybir.ActivationFunctionType.Exp)

        # persistent across phases
        persist = top.enter_context(tc.tile_pool(name="persist", bufs=1))
        qbf = persist.tile([128, NT, TOK], BF16)
        phiq_sb = persist.tile([128, NT], F32)
        phik_sb = persist.tile([128, NT], F32)

        # ---------------- Phase A ----------------
        with (
            tc.tile_pool(name="xin", bufs=1) as xin,
            tc.tile_pool(name="wts", bufs=24) as wts,
            tc.tile_pool(name="pa_mm", bufs=2, space="PSUM") as pa_mm,
            tc.tile_pool(name="pa_ss", bufs=2, space="PSUM") as pa_ss,
            tc.tile_pool(name="pa_g", bufs=1, space="PSUM") as pa_g,
            tc.tile_pool(name="raw", bufs=1) as rawp,
            tc.tile_pool(name="sqp", bufs=3) as sqp,
            tc.tile_pool(name="rope", bufs=3) as ropep,
            tc.tile_pool(name="outbf", bufs=3) as outbf,
            tc.tile_pool(name="small", bufs=2) as smallp,
            tc.tile_pool(name="frqp", bufs=2) as frqp,
        ):
            xT_sb = xin.tile([128, NT, TOK], BF16)

            def v_proj():
                # natural layout [tok, ch]; emitted between k and q proj so
                # its matmuls overlap the k rope tail on DVE. The first weight
                # block's loads interleave with the xT loads so matmul k can
                # start as soon as (xT[k], wv0[k]) land.
                for nb in range(4):
                    wv_nb = []
                    for k in range(NT):
                        if nb == 0:
                            nc.sync.dma_start(out=xT_sb[:, k, :], in_=xT.ap()[k])
                        wt = wts.tile([128, 384], BF16, tag="wt",
                                      name=f"wv{nb}_{k}")
                        nc.sync.dma_start(
                            out=wt, in_=wvT.ap()[k, :, nb * 384:(nb + 1) * 384])
                        wv_nb.append(wt)
                    for tb in range(7):
                        m = 128 if tb < 6 else MTAIL
                        pv = pa_mm.tile([128, 384], F32, tag="pmm",
                                        name=f"pv{nb}_{tb}")
                        for k in range(NT):
                            nc.tensor.matmul(
                                pv[:m, :],
                                lhsT=xT_sb[:, k, tb * 128: tb * 128 + m],
                                rhs=wv_nb[k],
                                start=(k == 0), stop=(k == NT - 1),
                            )
                        vbf = outbf.tile([128, 384], BF16, tag="vbf")
                        nc.scalar.copy(out=vbf[:m, :], in_=pv[:m, :])
                        nc.sync.dma_start(
                            out=v_in_view[tb * 128: tb * 128 + m,
                                          nb * 384:(nb + 1) * 384],
                            in_=vbf[:m, :],
                        )

            # ---- q/k projections (transposed layout [ch, tok]) ----
            QSA = [(0, 512), (512, TOK - 512)]

            def qk_proj(wdram, is_q):
                tagq = "q" if is_q else "k"
                raw = rawp.tile([128, NT, TOK], BF16, tag=f"raw{tagq}")
                psss = [pa_ss.tile([1, 512], F32, tag="pss", name=f"pss{tagq}{i}")
                        for i in range(2)]
                for half in range(4):
                    w_half = []
                    for k in range(NT):
                        wt = wts.tile([128, 384], BF16, tag="wt",
                                      name=f"w{tagq}{half}_{k}")
                        nc.sync.dma_start(
                            out=wt, in_=wdram.ap()[k, :, half * 384:(half + 1) * 384])
                        w_half.append(wt)
                    for oi in range(3):
                        ot = half * 3 + oi
                        pk = pa_mm.tile([128, TOK], F32, tag="pmm")
                        for qi, (q0, qn) in enumerate(QSA):
                            for k in range(NT):
                                nc.tensor.matmul(
                                    pk[:, q0:q0 + qn],
                                    lhsT=w_half[k][:, oi * 128:(oi + 1) * 128],
                                    rhs=xT_sb[:, k, q0:q0 + qn],
                                    start=(k == 0), stop=(k == NT - 1),
                                )
                        nc.scalar.copy(out=raw[:, ot, :], in_=pk)
                        sq = sqp.tile([128, TOK], BF16, tag="sq")
                        nc.vector.tensor_tensor(sq, raw[:, ot, :], raw[:, ot, :],
                                                mybir.AluOpType.mult)
                        for qi, (q0, qn) in enumerate(QSA):
                            nc.tensor.matmul(psss[qi][:, :qn], lhsT=ones_col_bf,
                                             rhs=sq[:, q0:q0 + qn],
                                             start=(ot == 0), stop=(ot == NT - 1))
                rs = smallp.tile([1, TOK], F32R, tag="rs")
                for qi, (q0, qn) in enumerate(QSA):
                    rs1 = smallp.tile([1, 512], F32, tag="rs1")
                    nc.scalar.activation(out=rs1[:, :qn], in_=psss[qi][:, :qn],
                                         func=mybir.ActivationFunctionType.Sqrt,
                                         bias=eps_sb[0:1, 0:1], scale=1.0 / D)
                    with nc.allow_low_precision(reason="rms scale in f32r"):
                        nc.vector.reciprocal(out=rs[:, q0:q0 + qn], in_=rs1[:, :qn])
                # broadcast rs to 128 partitions, fold into the rope tables
                prs = pa_g.tile([128, TOK], F32, tag="pg", name=f"prs{tagq}")
                for qi, (q0, qn) in enumerate(QSA):
                    nc.tensor.matmul(prs[:, q0:q0 + qn], lhsT=ones_row,
                                     rhs=rs[:, q0:q0 + qn], start=True, stop=True)
                prs_bf = smallp.tile([128, TOK], BF16, tag="prsbf")
                nc.scalar.copy(out=prs_bf, in_=prs)
                G1 = frqp.tile([128, TOK], BF16, tag="G1")
                nc.vector.tensor_tensor(G1, f11_sb, prs_bf, mybir.AluOpType.mult)
                G2 = frqp.tile([128, TOK], BF16, tag="G2")
                nc.vector.tensor_tensor(G2, f2n_sb, prs_bf, mybir.AluOpType.mult)
                for ot in range(NT):
                    # rope: pairs de-interleaved -> a=rows 0:C, b=rows C:128;
                    # dst = raw*[fr;fr] + swap(raw)*[-fi;fi], swap via DMA
                    rsw = ropep.tile([128, TOK], BF16, tag="rsw", bufs=12)
                    nc.sync.dma_start(out=rsw[0:C, :], in_=raw[C:128, ot, :])
                    nc.sync.dma_start(out=rsw[C:128, :], in_=raw[0:C, ot, :])
                    m1 = ropep.tile([128, TOK], BF16, tag="m1")
                    nc.vector.tensor_tensor(m1, raw[:, ot, :], G1,
                                            mybir.AluOpType.mult)
                    m2 = ropep.tile([128, TOK], BF16, tag="m2")
                    nc.vector.tensor_tensor(m2, rsw, G2,
                                            mybir.AluOpType.mult)
                    if is_q:
                        dst = qbf[:, ot, :]
                    else:
                        kbf = outbf.tile([128, TOK], BF16, tag="kbf")
                        dst = kbf[:, :]
                    phi_dst = phiq_sb if is_q else phik_sb
                    nc.vector.scalar_tensor_tensor(
                        out=dst, in0=m1, scalar=1.0, in1=m2,
                        op0=mybir.AluOpType.mult, op1=mybir.AluOpType.add,
                        accum_out=phi_dst[:, ot: ot + 1])
                    if not is_q:
                        nc.sync.dma_start(out=k_in_view[ot, :, 0:TOK], in_=kbf)

            v_proj()
            qk_proj(wkT, is_q=False)

            # AllGather (kT, v) once k and v blocks are written
            if not solo:
                nc.gpsimd.collective_compute(
                    "AllGather", mybir.AluOpType.bypass,
                    replica_groups=[list(range(n_cores))],
                    ins=[ag_in.ap().opt()], outs=[ag_out.ap().opt()],
                )

            qk_proj(wqT, is_q=True)

            # ---- phi AllReduce ----
            nc.sync.dma_start(out=phi_in.ap()[:, :, 0:1],
                              in_=phiq_sb[:, :, None])
            phik_m = smallp.tile([128, NT, NCH], F32, tag="phikm")
            for ch in range(NCH):
                nc.vector.tensor_scalar_mul(phik_m[:, :, ch], phik_sb,
                                            cm_sb[:, ch: ch + 1])
            nc.sync.dma_start(out=phi_in.ap()[:, :, 1: 1 + NCH], in_=phik_m)
            if not solo:
                nc.gpsimd.collective_compute(
                    "AllReduce", mybir.AluOpType.add,
                    replica_groups=[list(range(n_cores))],
                    ins=[phi_in.ap().opt()], outs=[phi_out.ap().opt()],
                )

            # ---- routing scores + top-2 chunk indices ----
            phis = smallp.tile([128, NT, 1 + NCH], F32, tag="phis")
            nc.sync.dma_start(out=phis, in_=(phi_in if solo else phi_out).ap())
            prod = smallp.tile([128, NT, NCH], F32R, tag="prodsc")
            nc.vector.tensor_tensor(
                prod, phis[:, :, 1: 1 + NCH],
                phis[:, :, 0:1].to_broadcast((128, NT, NCH)),
                mybir.AluOpType.mult)
            psc = pa_ss.tile([1, NH * NCH], F32, tag="pss", name="psc")
            nc.tensor.matmul(psc, lhsT=ones_col,
                             rhs=prod[:, :, :].rearrange("p t c -> p (t c)"),
                             start=True, stop=True)
            sc = smallp.tile([1, NH * NCH], F32, tag="sc")
            nc.vector.tensor_copy(out=sc, in_=psc)
            scv = sc[:, :].rearrange("p (h c) -> p h c", c=NCH)
            m1s = smallp.tile([1, NH], F32, tag="m1s")
            nc.vector.reduce_max(out=m1s, in_=scv, axis=mybir.AxisListType.X)
            is1 = smallp.tile([1, NH * NCH], F32, tag="is1")
            nc.vector.tensor_tensor(
                is1[:, :].rearrange("p (h c) -> p h c", c=NCH),
                scv, m1s[:, :, None].to_broadcast((1, NH, NCH)),
                mybir.AluOpType.is_ge)
            nc.vector.tensor_scalar_mul(is1, is1, 1e30)
            masked = smallp.tile([1, NH * NCH], F32, tag="masked")
            nc.vector.tensor_tensor(masked, sc, is1, mybir.AluOpType.subtract)
            m2s = smallp.tile([1, NH], F32, tag="m2s")
            nc.vector.reduce_max(out=m2s,
                                 in_=masked[:, :].rearrange("p (h c) -> p h c", c=NCH),
                                 axis=mybir.AxisListType.X)
            # chunk indices: i1 = argmax, i2 = arg-2nd-max (per head)
            iota4 = smallp.tile([1, NCH], F32, tag="iota4")
            nc.gpsimd.iota(iota4.bitcast(mybir.dt.int32), pattern=[[1, NCH]],
                           base=0, channel_multiplier=0)
            nc.vector.tensor_copy(out=iota4, in_=iota4.bitcast(mybir.dt.int32))
            is2 = smallp.tile([1, NH * NCH], F32, tag="is2")
            nc.vector.tensor_tensor(
                is2[:, :].rearrange("p (h c) -> p h c", c=NCH),
                masked[:, :].rearrange("p (h c) -> p h c", c=NCH),
                m2s[:, :, None].to_broadcast((1, NH, NCH)),
                mybir.AluOpType.is_ge)
            nc.vector.tensor_scalar_mul(is1, is1, 1e-30)  # undo 1e30 scale -> 0/1
            idxf = smallp.tile([1, NH, 2], F32, tag="idxf")
            w1 = smallp.tile([1, NH * NCH], F32, tag="w1")
            nc.vector.tensor_tensor(
                w1[:, :].rearrange("p (h c) -> p h c", c=NCH),
                is1[:, :].rearrange("p (h c) -> p h c", c=NCH),
                iota4[:, None, :].to_broadcast((1, NH, NCH)),
                mybir.AluOpType.mult)
            nc.vector.reduce_sum(out=idxf[:, :, 0], in_=w1[:, :].rearrange(
                "p (h c) -> p h c", c=NCH), axis=mybir.AxisListType.X)
            nc.vector.tensor_tensor(
                w1[:, :].rearrange("p (h c) -> p h c", c=NCH),
                is2[:, :].rearrange("p (h c) -> p h c", c=NCH),
                iota4[:, None, :].to_broadcast((1, NH, NCH)),
                mybir.AluOpType.mult)
            nc.vector.reduce_sum(out=idxf[:, :, 1], in_=w1[:, :].rearrange(
                "p (h c) -> p h c", c=NCH), axis=mybir.AxisListType.X)
            idx_i32 = persist.tile([1, NH * 2], mybir.dt.int32)
            nc.vector.tensor_copy(out=idx_i32,
                                  in_=idxf[:, :, :].rearrange("p h s -> p (h s)"))

        # ---------------- Phase B: attention ----------------
        otp = top.enter_context(tc.tile_pool(name="otp", bufs=1))
        oT_sb = otp.tile([128, NT, TOK], BF16)
        if phases == "a":
            return _finish(nc)
        QS = [(0, 512), (512, TOK - 512)]  # bank-aligned query splits
        wop = top.enter_context(tc.tile_pool(name="wo", bufs=12))
        from concourse.bass import ds as _ds
        with (
            tc.tile_pool(name="kv", bufs=10) as kvp,
            tc.tile_pool(name="ktl", bufs=2) as ktlp,
            tc.tile_pool(name="ebf", bufs=3) as ep,
            tc.tile_pool(name="accp", bufs=6) as accp,
            tc.tile_pool(name="dacc", bufs=2) as dp,
            tc.tile_pool(name="bsm", bufs=2) as bsm,
            tc.tile_pool(name="pb_s", bufs=2, space="PSUM") as pb_s,
            tc.tile_pool(name="pb_o", bufs=4, space="PSUM") as pb_o,
        ):
            # flat software-pipelined stream over all (head, kv-tile) steps:
            # the logits matmul for step i+1 is emitted BEFORE step i's PV so
            # the in-order PE queue never head-blocks on the exp semaphore.
            n_mm = 4 * NKT + 1
            steps = [(h, bi, kt) for h in range(NH)
                     for bi, kt in ([(b, k) for b in range(4)
                                     for k in range(NKT)] + [(4, 0)])]
            st = {}

            def head_setup(h):
                pos = [pb_o.tile([128, 512], F32, tag="po", name=f"po{h}_{qb}")
                       for qb in range(2)]
                blk_regs = []
                for sel in range(2):
                    iv = nc.values_load(
                        idx_i32[0:1, h * 2 + sel: h * 2 + sel + 1],
                        min_val=0, max_val=NCH - 1,
                        skip_runtime_bounds_check=True)
                    blk_regs.append(iv)
                ktail = ktlp.tile([128, NTAIL], BF16, tag="ktail")
                vtail = ktlp.tile([NTAIL, HD], BF16, tag="vtail")
                kts, vbs = [], []
                for bi in range(4):
                    blk = blk_regs[bi // 2] * 2 + (bi % 2)
                    kv_b = (ag_in.ap() if solo
                            else ag_out.ap()[_ds(blk, 1)][0])
                    kT_b = kvp.tile([128, TOK], BF16, tag="kb",
                                    name=f"kT{h}_{bi}")
                    k_view = kv_b[0].rearrange("(t p n) -> t p n",
                                               p=128, n=TOK)[h]
                    nc.sync.dma_start(out=kT_b, in_=k_view)
                    nc.sync.dma_start(
                        out=ktail[:, bi * KTAIL:(bi + 1) * KTAIL],
                        in_=k_view[:, NKT * 128: TOK])
                    V_b = kvp.tile([128, NKT, HD], BF16, tag="vb",
                                   name=f"V{h}_{bi}")
                    nc.sync.dma_start(
                        out=V_b,
                        in_=kv_b[1][0:NKT * 128 * D]
                        .rearrange("(n p d) -> p n d", p=128, d=D)
                        [:, :, h * HD:(h + 1) * HD])
                    nc.sync.dma_start(
                        out=vtail[bi * KTAIL:(bi + 1) * KTAIL, :],
                        in_=kv_b[1].rearrange("(t d) -> t d", d=D)
                        [NKT * 128: TOK, h * HD:(h + 1) * HD])
                    kts.append(kT_b)
                    vbs.append(V_b)
                st[h] = dict(pos=pos, ktail=ktail, vtail=vtail, kts=kts,
                             vbs=vbs, ebs={}, accs=[], ps={}, etail=None)

            def emit_ps(i):
                h, bi, kt = steps[i]
                if (bi, kt) == (0, 0) and h not in st:
                    head_setup(h)
                s = st[h]
                ps = pb_s.tile([128, TOK], F32, tag="ps", name=f"ps{i}")
                if bi == 4:
                    lhs, rows = s["ktail"][:, :], NTAIL
                else:
                    lhs, rows = s["kts"][bi][:, kt * 128:(kt + 1) * 128], 128
                for qb, (q0, qn) in enumerate(QS):
                    nc.tensor.matmul(ps[0:rows, q0:q0 + qn], lhsT=lhs,
                                     rhs=qbf[:, h, q0:q0 + qn],
                                     start=True, stop=True)
                s["ps"][(bi, kt)] = ps

            pending = []

            def emit_midA(h, dr):
                # denominator partition-reduce, first query half only: the
                # two pdp matmuls are split across two drain slots so each
                # displaces the logits stream by half as much
                pdp = pb_s.tile([1, TOK], F32, tag="ps", name=f"pdp{h}")
                q0, qn = QS[0]
                nc.tensor.matmul(pdp[:, q0:q0 + qn], lhsT=ones_col_bf,
                                 rhs=dr[:, q0:q0 + qn], start=True, stop=True)
                return pdp

            def emit_midB(h, dr, pdp):
                q0, qn = QS[1]
                nc.tensor.matmul(pdp[:, q0:q0 + qn], lhsT=ones_col_bf,
                                 rhs=dr[:, q0:q0 + qn], start=True, stop=True)
                rec = bsm.tile([1, TOK], BF16, tag="rec", name=f"rec{h}")
                with nc.allow_low_precision(reason="softmax denom"):
                    nc.vector.reciprocal(out=rec, in_=pdp)
                # DRAM round-trip broadcast: replicate the reciprocal row to
                # 128 partitions on the DMA engines (keeps PE out of the
                # per-head tail entirely)
                nc.sync.dma_start(out=rec_dram.ap()[h:h + 1, :], in_=rec)
                return rec

            def emit_finalize(h, rec, pos):
                rb_sb = bsm.tile([128, TOK], BF16, tag="rbsb")
                nc.sync.dma_start(
                    out=rb_sb,
                    in_=rec_dram.ap()[h:h + 1, :].to_broadcast((128, TOK)))
                for qb, (q0, qn) in enumerate(QS):
                    po_bf = bsm.tile([128, 512], BF16, tag="pobf",
                                     name=f"pobf{h}_{qb}")
                    nc.vector.tensor_copy(out=po_bf[:, :qn],
                                          in_=pos[qb][:, :qn])
                    nc.vector.tensor_tensor(oT_sb[:, h, q0:q0 + qn],
                                            po_bf[:, :qn],
                                            rb_sb[:, q0:q0 + qn],
                                            mybir.AluOpType.mult)

            def drain(i):
                while pending and pending[0][0] <= i:
                    _, kind, fh, fdat, fpos = pending.pop(0)
                    if kind == "midA":
                        pdp = emit_midA(fh, fdat)
                        pending.append((i + 1, "midB", fh, (fdat, pdp), fpos))
                        pending.sort(key=lambda e: e[0])
                    elif kind == "midB":
                        rec = emit_midB(fh, fdat[0], fdat[1])
                        pending.append((i + 3, "fin", fh, rec, fpos))
                        pending.sort(key=lambda e: e[0])
                    else:
                        emit_finalize(fh, fdat, fpos)

            emit_ps(0)
            emit_ps(1)
            wo_tiles = []
            for i, (h, bi, kt) in enumerate(steps):
                if i + 2 < len(steps):
                    emit_ps(i + 2)
                # prefetch the NEXT head's kv DMAs well ahead of its first
                # logits so the boundary exp never waits on the gather
                j = i + 6
                if j < len(steps) and steps[j][1:] == (0, 0) \
                        and steps[j][0] not in st:
                    head_setup(steps[j][0])
                drain(i)
                if i == 30:
                    # wo loads issued mid-attention: off the boundary-critical
                    # DMA queue, done long before the output projection
                    for k in range(NT):
                        wt = wop.tile([128, D], BF16, tag="wo", name=f"wo{k}")
                        nc.sync.dma_start(out=wt, in_=woT.ap()[k])
                        wo_tiles.append(wt)
                s = st[h]
                ps = s["ps"].pop((bi, kt))
                if bi == 4:
                    etail = bsm.tile([NTAIL, TOK], BF16, tag="etail")
                    nc.scalar.activation(out=etail, in_=ps[0:NTAIL, :],
                                         func=mybir.ActivationFunctionType.Exp,
                                         scale=SM_SCALE)
                    s["etail"] = etail
                    e_ap, lhsT_pv, i_mm = etail[:, :], s["vtail"][:, :], 4 * NKT
                else:
                    if kt == 0:
                        s["ebs"][bi] = ep.tile([128, NKT, TOK], BF16, tag="eb",
                                               name=f"eb{h}_{bi}")
                    eb = s["ebs"][bi]
                    nc.scalar.activation(out=eb[:, kt, :], in_=ps,
                                         func=mybir.ActivationFunctionType.Exp,
                                         scale=SM_SCALE)
                    e_ap, lhsT_pv = eb[:, kt, :], s["vbs"][bi][:, kt, :]
                    i_mm = bi * NKT + kt
                for qb, (q0, qn) in enumerate(QS):
                    nc.tensor.matmul(s["pos"][qb][:, :qn], lhsT=lhsT_pv,
                                     rhs=e_ap[:, q0:q0 + qn],
                                     start=(i_mm == 0), stop=(i_mm == n_mm - 1))
                if bi < 4 and kt == NKT - 1:
                    # per-block pairwise denominator tree (bf16, 2x DVE)
                    eb = s["ebs"][bi]
                    t1 = accp.tile([128, TOK], BF16, tag="dt", name="t1")
                    nc.vector.tensor_tensor(t1, eb[:, 0, :], eb[:, 1, :],
                                            mybir.AluOpType.add)
                    t2 = accp.tile([128, TOK], BF16, tag="dt", name="t2")
                    nc.vector.tensor_tensor(t2, eb[:, 2, :], eb[:, 3, :],
                                            mybir.AluOpType.add)
                    t3 = accp.tile([128, TOK], BF16, tag="dt", name="t3")
                    nc.vector.tensor_tensor(t3, eb[:, 4, :], eb[:, 5, :],
                                            mybir.AluOpType.add)
                    nc.vector.tensor_tensor(t1, t1, t2, mybir.AluOpType.add)
                    acc = accp.tile([128, TOK], BF16, tag="acc",
                                    name=f"acc{h}_{bi}")
                    nc.vector.tensor_tensor(acc, t1, t3, mybir.AluOpType.add)
                    s["accs"].append(acc)
                if bi == 4:
                    # den = (acc0+acc1+tail) + (acc2+acc3); off the PE queue
                    accs, etail = s["accs"], s["etail"]
                    s01 = accp.tile([128, TOK], BF16, tag="dt", name="s01")
                    nc.vector.tensor_tensor(s01, accs[0], accs[1],
                                            mybir.AluOpType.add)
                    nc.vector.tensor_tensor(s01[0:NTAIL, :], s01[0:NTAIL, :],
                                            etail, mybir.AluOpType.add)
                    s23 = accp.tile([128, TOK], BF16, tag="dt", name="s23")
                    nc.vector.tensor_tensor(s23, accs[2], accs[3],
                                            mybir.AluOpType.add)
                    dr = dp.tile([128, TOK], BF16, tag="dr", name=f"dr{h}")
                    nc.vector.tensor_tensor(dr, s01, s23, mybir.AluOpType.add)
                    pending.append((i + 5, "midA", h, dr, s["pos"]))
                    pending.sort(key=lambda e: e[0])
                    del st[h]
            while pending:
                _, kind, fh, fdat, fpos = pending.pop(0)
                if kind == "midA":
                    pdp = emit_midA(fh, fdat)
                    rec = emit_midB(fh, fdat, pdp)
                    emit_finalize(fh, rec, fpos)
                elif kind == "midB":
                    rec = emit_midB(fh, fdat[0], fdat[1])
                    emit_finalize(fh, rec, fpos)
                else:
                    emit_finalize(fh, fdat, fpos)

        # ---------------- out projection ----------------
        if phases == "ab":
            return _finish(nc)
        with (
            tc.tile_pool(name="osb", bufs=3) as osb,
            tc.tile_pool(name="po_mm", bufs=2, space="PSUM") as po_mm,
        ):
            groups = [(tb, nb) for tb in range(7) for nb in range(3)]

            def og_mm(pO, tb, nb, m, ks):
                for k in ks:
                    nc.tensor.matmul(pO[:m, :],
                                     lhsT=oT_sb[:, k, tb * 128: tb * 128 + m],
                                     rhs=wo_tiles[k][:, nb * 512:(nb + 1) * 512],
                                     start=(k == 0), stop=(k == NT - 1))

            def og_out(pO, tb, nb, m):
                ob = osb.tile([128, 512], F32, tag="ob")
                nc.scalar.copy(out=ob[:m, :], in_=pO[:m, :])
                nc.sync.dma_start(
                    out=out.ap()[tb * 128: tb * 128 + m,
                                 nb * 512:(nb + 1) * 512],
                    in_=ob[:m, :])

            # first two groups interleave: their head-11 matmuls come after
            # 22 ready matmuls so PE doesn't head-block on the last head's
            # deferred finalize writing oT[:, 11, :]
            p0 = po_mm.tile([128, 512], F32, tag="pO", name="pO0")
            og_mm(p0, 0, 0, 128, range(NT - 1))
            p1 = po_mm.tile([128, 512], F32, tag="pO", name="pO1")
            og_mm(p1, 0, 1, 128, range(NT - 1))
            og_mm(p0, 0, 0, 128, [NT - 1])
            og_out(p0, 0, 0, 128)
            og_mm(p1, 0, 1, 128, [NT - 1])
            og_out(p1, 0, 1, 128)
            for tb, nb in groups[2:]:
                m = 128 if tb < 6 else MTAIL
                pO = po_mm.tile([128, 512], F32, tag="pO")
                og_mm(pO, tb, nb, m, range(NT))
                og_out(pO, tb, nb, m)

    return _finish(nc)


def _finish(nc):
    nc.compile()
    return nc


# ---------------- host-side prep ----------------

def _perm():
    p = np.arange(D).reshape(NH, C, 2)
    return np.concatenate([p[:, :, 0], p[:, :, 1]], axis=1).reshape(-1)


def make_fcis(freqs, grid_sizes):
    f, h, w = [int(v) for v in np.asarray(grid_sizes)[0]]
    c1 = C - 2 * (C // 3)
    c2 = C // 3
    fq = np.asarray(freqs, np.float32)
    ff = np.broadcast_to(fq[:f, None, None, :c1], (f, h, w, c1, 2))
    fh = np.broadcast_to(fq[None, :h, None, c1:c1 + c2], (f, h, w, c2, 2))
    fw = np.broadcast_to(fq[None, None, :w, c1 + c2:c1 + 2 * c2], (f, h, w, c2, 2))
    fcis = np.concatenate([ff, fh, fw], axis=3).reshape(f * h * w, C, 2)
    if fcis.shape[0] == 1 and S > 1:
        fcis = np.broadcast_to(fcis, (S, C, 2))
    return fcis


def host_prep(inputs):
    """inputs: the full reference input dict -> per-core in_maps."""
    import ml_dtypes
    bf16 = ml_dtypes.bfloat16
    x = np.asarray(inputs["x"], np.float32)
    freqs = np.asarray(inputs["freqs"], np.float32)
    grid_sizes = np.asarray(inputs["grid_sizes"])
    assert x.shape == (1, S, D)
    assert int(np.asarray(inputs["chunk_size"])) == S // NCH
    assert int(np.asarray(inputs["top_k"])) == 2

    perm = _perm()
    wq = np.asarray(inputs["wq"], np.float32)[perm]
    wk = np.asarray(inputs["wk"], np.float32)[perm]
    wv = np.asarray(inputs["wv"], np.float32)
    wo = np.asarray(inputs["wo"], np.float32)
    for b in ("bq", "bk", "bv", "bo"):
        assert not np.any(np.asarray(inputs[b])), f"nonzero bias {b} unsupported"
    for g in ("gq", "gk"):
        assert np.all(np.asarray(inputs[g]) == 1.0), f"non-unit gain {g} unsupported"

    xT = np.ascontiguousarray(x[0].T).reshape(NT, 128, S).astype(bf16)
    wqT = np.ascontiguousarray(wq.T).reshape(NT, 128, D).astype(bf16)
    wkT = np.ascontiguousarray(wk.T).reshape(NT, 128, D).astype(bf16)
    wvT = np.ascontiguousarray(wv.T).reshape(NT, 128, D).astype(bf16)
    woT = np.ascontiguousarray(wo.T).reshape(NT, 128, D).astype(bf16)

    fcis = make_fcis(freqs, grid_sizes)  # [S, C, 2]
    frT = fcis[:, :, 0].T  # [C, S]
    fiT = fcis[:, :, 1].T
    f11 = np.concatenate([frT, frT], axis=0)  # [128, S]
    f2n = np.concatenate([-fiT, fiT], axis=0)

    in_maps = []
    for c in range(N_CORES):
        sl = slice(c * TOK, (c + 1) * TOK)
        cm = np.zeros((128, NCH), np.float32)
        cm[:, (c * TOK) // (S // NCH)] = 1.0
        in_maps.append({
            "xT": np.ascontiguousarray(xT[:, :, sl]),
            "wqT": wqT, "wkT": wkT, "wvT": wvT, "woT": woT,
            "f11": np.ascontiguousarray(f11[:, sl]).astype(bf16),
            "f2n": np.ascontiguousarray(f2n[:, sl]).astype(bf16),
            "chmask": cm,
        })
    return in_maps


def assemble_out(results):
    return np.concatenate([r["out"] for r in results], axis=0)[None]


# ---------------- harness entry point ----------------

_CACHE = {}


def kernel(**inputs):
    import numpy as _np
    key = "nc"
    if key not in _CACHE:
        _CACHE[key] = build_kernel()
    nc = _CACHE[key]
    in_maps = host_prep(inputs)
    from concourse import bass_utils
    res = bass_utils.run_bass_kernel_spmd(
        nc, in_maps, core_ids=list(range(N_CORES)), trace=False)
    return assemble_out(res.results).astype(_np.float32)


# revision 77
# speedup vs baseline: 1.0024x; 1.0024x over previous
"""nn_CausalWanSelfAttention Trainium2 kernel (8-core SPMD, single launch).

Entry point: kernel(**inputs) -> np.ndarray [1, 6240, 1536] float32.

Strategy (v2 — bf16 datapath, packed kv, software-pipelined attention):
  - Phase A, token-sharded (780 tokens/core): q/k/v projections as bf16
    matmuls (f32 PSUM accumulation), rmsnorm via bf16 square + ones-matmul
    partition reduction. 3D-RoPE on pair-de-interleaved channels: the
    partition-half swap runs on the DMA engines (SBUF->SBUF), then
    dst = raw*[fr;fr] + swap(raw)*[-fi;fi] as same-partition bf16 ops
    (2x DVE mode); the final add fuses the phi token-reduction via
    scalar_tensor_tensor(accum_out=...). The per-token rmsnorm scale
    folds into the rope tables once per projection.
  - One AllGather ships (k^T, v) in bf16 (780 tokens per block, no pad);
    one small AllReduce combines the routing means phi_q/phi_k.
  - Top-2-of-4 chunk routing on device; per-head chunk indices drive
    dynamically-addressed DMA gathers of the selected K/V blocks.
  - Phase B, query-sharded, emitted as ONE flat software-pipelined stream
    over all (head, kv-tile) steps: the logits matmul for step i+1 is
    emitted before step i's PV so the in-order PE queue never head-blocks
    on the exp semaphore, and each head's denominator reduce / reciprocal
    / o^T normalization are deferred several steps into the next head.
    24 full 128-row kv tiles plus ONE packed tail tile assembled from the
    4 selected blocks' 12 leftover rows (25 tile-ops per engine, not 28).
    exp on the scalar engine (the pacing engine, ~87% busy); softmax
    denominator via a bf16 pairwise tree on the vector engine; PV
    accumulation in PSUM; the row-parallel output projection reads o^T
    (bf16) from SBUF.
"""

from contextlib import ExitStack

import numpy as np

import concourse.bacc as bacc
import concourse.mybir as mybir
import concourse.tile as tile

F32R = mybir.dt.float32r
F32 = mybir.dt.float32
BF16 = mybir.dt.bfloat16
KV_BYTES_PER_ELEM = 2    # K/V ship over the AllGather in bf16

N_CORES = 8
S, D, NH, HD, C = 6240, 1536, 12, 128, 64
NT = D // 128          # 12 channel tiles
TOK = S // N_CORES     # 780 tokens per core
NBLK = N_CORES         # 8 kv blocks (one per core)
NCH = 4                # routing chunks
NKT = 6                # full 128-row kv tiles per block (768 rows)
KTAIL = TOK - NKT * 128   # 12 leftover kv rows per block
NTAIL = 4 * KTAIL         # 48 packed tail rows (4 selected blocks)
EPS = 1e-6
SM_SCALE = 1.0 / float(np.sqrt(HD))
KV_ELEMS = NT * 128 * TOK  # bf16 elements per (k or v) per core
MTAIL = TOK - 6 * 128  # 12 (token tail within a core block)


def build_kernel(n_cores=N_CORES, solo=False, phases="full"):
    nc = bacc.Bacc("TRN2", target_bir_lowering=False, debug=False,
                   num_devices=n_cores)

    xT = nc.dram_tensor("xT", [NT, 128, TOK], BF16, kind="ExternalInput")
    wqT = nc.dram_tensor("wqT", [NT, 128, D], BF16, kind="ExternalInput")
    wkT = nc.dram_tensor("wkT", [NT, 128, D], BF16, kind="ExternalInput")
    wvT = nc.dram_tensor("wvT", [NT, 128, D], BF16, kind="ExternalInput")
    woT = nc.dram_tensor("woT", [NT, 128, D], BF16, kind="ExternalInput")
    f11 = nc.dram_tensor("f11", [128, TOK], BF16, kind="ExternalInput")
    f2n = nc.dram_tensor("f2n", [128, TOK], BF16, kind="ExternalInput")
    chmask = nc.dram_tensor("chmask", [128, NCH], F32, kind="ExternalInput")

    out = nc.dram_tensor("out", [TOK, D], F32, kind="ExternalOutput")

    # collective buffers
    ag_in = nc.dram_tensor("ag_in", [2, KV_ELEMS], BF16)
    ag_out = nc.dram_tensor("ag_out", [NBLK, 2, KV_ELEMS], BF16,
                            addr_space="Shared")
    phi_in = nc.dram_tensor("phi_in", [128, NT, 1 + NCH], F32)
    phi_out = nc.dram_tensor("phi_out", [128, NT, 1 + NCH], F32,
                             addr_space="Shared")
    rec_dram = nc.dram_tensor("rec_dram", [NH, TOK], BF16)

    k_in_view = ag_in.ap()[0].rearrange("(t p n) -> t p n", p=128, n=TOK)
    v_in_view = ag_in.ap()[1].rearrange("(k d) -> k d", d=D)

    ones_col_t = nc.inline_tensor(np.ones((128, 1), np.float32), name="ones_col")
    ones_row_t = nc.inline_tensor(np.ones((1, 128), np.float32), name="ones_row")

    with tile.TileContext(nc) as tc, ExitStack() as top:
        consts = top.enter_context(tc.tile_pool(name="consts", bufs=1))
        ones_col = consts.tile([128, 1], F32R)
        nc.sync.dma_start(out=ones_col, in_=ones_col_t.ap().bitcast(F32R))
        ones_row = consts.tile([1, 128], F32R)
        nc.sync.dma_start(out=ones_row, in_=ones_row_t.ap().bitcast(F32R))
        ones_col_bf = consts.tile([128, 1], BF16)
        nc.vector.memset(ones_col_bf, 1.0)
        f11_sb = consts.tile([128, TOK], BF16)
        nc.sync.dma_start(out=f11_sb, in_=f11[:, :])
        f2n_sb = consts.tile([128, TOK], BF16)
        nc.sync.dma_start(out=f2n_sb, in_=f2n[:, :])
        cm_sb = consts.tile([128, NCH], F32)
        nc.sync.dma_start(out=cm_sb, in_=chmask[:, :])
        eps_sb = consts.tile([1, 1], F32)
        nc.vector.memset(eps_sb, EPS)
        # preload the Exp activation table so the first attention exp
        # doesn't pay the 1.3us LoadActFuncSet at the phase boundary
        warm = consts.tile([1, 1], F32)
        nc.scalar.activation(out=warm, in_=eps_sb,
                             func=mybir.ActivationFunctionType.Exp)

        # persistent across phases
        persist = top.enter_context(tc.tile_pool(name="persist", bufs=1))
        qbf = persist.tile([128, NT, TOK], BF16)
        phiq_sb = persist.tile([128, NT], F32)
        phik_sb = persist.tile([128, NT], F32)

        # ---------------- Phase A ----------------
        with (
            tc.tile_pool(name="xin", bufs=1) as xin,
            tc.tile_pool(name="wts", bufs=24) as wts,
            tc.tile_pool(name="pa_mm", bufs=2, space="PSUM") as pa_mm,
            tc.tile_pool(name="pa_ss", bufs=2, space="PSUM") as pa_ss,
            tc.tile_pool(name="pa_g", bufs=1, space="PSUM") as pa_g,
            tc.tile_pool(name="raw", bufs=1) as rawp,
            tc.tile_pool(name="sqp", bufs=3) as sqp,
            tc.tile_pool(name="rope", bufs=3) as ropep,
            tc.tile_pool(name="outbf", bufs=3) as outbf,
            tc.tile_pool(name="small", bufs=2) as smallp,
            tc.tile_pool(name="frqp", bufs=2) as frqp,
        ):
            xT_sb = xin.tile([128, NT, TOK], BF16)

            def v_proj():
                # natural layout [tok, ch]; emitted between k and q proj so
                # its matmuls overlap the k rope tail on DVE. The first weight
                # block's loads interleave with the xT loads so matmul k can
                # start as soon as (xT[k], wv0[k]) land.
                for nb in range(4):
                    wv_nb = []
                    for k in range(NT):
                        if nb == 0:
                            nc.sync.dma_start(out=xT_sb[:, k, :], in_=xT.ap()[k])
                        wt = wts.tile([128, 384], BF16, tag="wt",
                                      name=f"wv{nb}_{k}")
                        nc.sync.dma_start(
                            out=wt, in_=wvT.ap()[k, :, nb * 384:(nb + 1) * 384])
                        wv_nb.append(wt)
                    for tb in range(7):
                        m = 128 if tb < 6 else MTAIL
                        pv = pa_mm.tile([128, 384], F32, tag="pmm",
                                        name=f"pv{nb}_{tb}")
                        for k in range(NT):
                            nc.tensor.matmul(
                                pv[:m, :],
                                lhsT=xT_sb[:, k, tb * 128: tb * 128 + m],
                                rhs=wv_nb[k],
                                start=(k == 0), stop=(k == NT - 1),
                            )
                        vbf = outbf.tile([128, 384], BF16, tag="vbf")
                        nc.scalar.copy(out=vbf[:m, :], in_=pv[:m, :])
                        nc.sync.dma_start(
                            out=v_in_view[tb * 128: tb * 128 + m,
                                          nb * 384:(nb + 1) * 384],
                            in_=vbf[:m, :],
                        )

            # ---- q/k projections (transposed layout [ch, tok]) ----
            QSA = [(0, 512), (512, TOK - 512)]

            def qk_proj(wdram, is_q):
                tagq = "q" if is_q else "k"
                raw = rawp.tile([128, NT, TOK], BF16, tag=f"raw{tagq}")
                psss = [pa_ss.tile([1, 512], F32, tag="pss", name=f"pss{tagq}{i}")
                        for i in range(2)]
                for half in range(4):
                    w_half = []
                    for k in range(NT):
                        wt = wts.tile([128, 384], BF16, tag="wt",
                                      name=f"w{tagq}{half}_{k}")
                        nc.sync.dma_start(
                            out=wt, in_=wdram.ap()[k, :, half * 384:(half + 1) * 384])
                        w_half.append(wt)
                    for oi in range(3):
                        ot = half * 3 + oi
                        pk = pa_mm.tile([128, TOK], F32, tag="pmm")
                        for qi, (q0, qn) in enumerate(QSA):
                            for k in range(NT):
                                nc.tensor.matmul(
                                    pk[:, q0:q0 + qn],
                                    lhsT=w_half[k][:, oi * 128:(oi + 1) * 128],
                                    rhs=xT_sb[:, k, q0:q0 + qn],
                                    start=(k == 0), stop=(k == NT - 1),
                                )
                        nc.scalar.copy(out=raw[:, ot, :], in_=pk)
                        sq = sqp.tile([128, TOK], BF16, tag="sq")
                        nc.vector.tensor_tensor(sq, raw[:, ot, :], raw[:, ot, :],
                                                mybir.AluOpType.mult)
                        for qi, (q0, qn) in enumerate(QSA):
                            nc.tensor.matmul(psss[qi][:, :qn], lhsT=ones_col_bf,
                                             rhs=sq[:, q0:q0 + qn],
                                             start=(ot == 0), stop=(ot == NT - 1))
                rs = smallp.tile([1, TOK], F32R, tag="rs")
                for qi, (q0, qn) in enumerate(QSA):
                    rs1 = smallp.tile([1, 512], F32, tag="rs1")
                    nc.scalar.activation(out=rs1[:, :qn], in_=psss[qi][:, :qn],
                                         func=mybir.ActivationFunctionType.Sqrt,
                                         bias=eps_sb[0:1, 0:1], scale=1.0 / D)
                    with nc.allow_low_precision(reason="rms scale in f32r"):
                        nc.vector.reciprocal(out=rs[:, q0:q0 + qn], in_=rs1[:, :qn])
                # broadcast rs to 128 partitions, fold into the rope tables
                prs = pa_g.tile([128, TOK], F32, tag="pg", name=f"prs{tagq}")
                for qi, (q0, qn) in enumerate(QSA):
                    nc.tensor.matmul(prs[:, q0:q0 + qn], lhsT=ones_row,
                                     rhs=rs[:, q0:q0 + qn], start=True, stop=True)
                prs_bf = smallp.tile([128, TOK], BF16, tag="prsbf")
                nc.scalar.copy(out=prs_bf, in_=prs)
                G1 = frqp.tile([128, TOK], BF16, tag="G1")
                nc.vector.tensor_tensor(G1, f11_sb, prs_bf, mybir.AluOpType.mult)
                G2 = frqp.tile([128, TOK], BF16, tag="G2")
                nc.vector.tensor_tensor(G2, f2n_sb, prs_bf, mybir.AluOpType.mult)
                for ot in range(NT):
                    # rope: pairs de-interleaved -> a=rows 0:C, b=rows C:128;
                    # dst = raw*[fr;fr] + swap(raw)*[-fi;fi], swap via DMA
                    rsw = ropep.tile([128, TOK], BF16, tag="rsw", bufs=12)
                    nc.sync.dma_start(out=rsw[0:C, :], in_=raw[C:128, ot, :])
                    nc.sync.dma_start(out=rsw[C:128, :], in_=raw[0:C, ot, :])
                    m1 = ropep.tile([128, TOK], BF16, tag="m1")
                    nc.vector.tensor_tensor(m1, raw[:, ot, :], G1,
                                            mybir.AluOpType.mult)
                    m2 = ropep.tile([128, TOK], BF16, tag="m2")
                    nc.vector.tensor_tensor(m2, rsw, G2,
                                            mybir.AluOpType.mult)
                    if is_q:
                        dst = qbf[:, ot, :]
                    else:
                        kbf = outbf.tile([128, TOK], BF16, tag="kbf")
                        dst = kbf[:, :]
                    phi_dst = phiq_sb if is_q else phik_sb
                    nc.vector.scalar_tensor_tensor(
                        out=dst, in0=m1, scalar=1.0, in1=m2,
                        op0=mybir.AluOpType.mult, op1=mybir.AluOpType.add,
                        accum_out=phi_dst[:, ot: ot + 1])
                    if not is_q:
                        nc.sync.dma_start(out=k_in_view[ot, :, 0:TOK], in_=kbf)

            v_proj()
            qk_proj(wkT, is_q=False)

            # AllGather (kT, v) once k and v blocks are written
            if not solo:
                nc.gpsimd.collective_compute(
                    "AllGather", mybir.AluOpType.bypass,
                    replica_groups=[list(range(n_cores))],
                    ins=[ag_in.ap().opt()], outs=[ag_out.ap().opt()],
                )

            qk_proj(wqT, is_q=True)

            # ---- phi AllReduce ----
            nc.sync.dma_start(out=phi_in.ap()[:, :, 0:1],
                              in_=phiq_sb[:, :, None])
            phik_m = smallp.tile([128, NT, NCH], F32, tag="phikm")
            for ch in range(NCH):
                nc.vector.tensor_scalar_mul(phik_m[:, :, ch], phik_sb,
                                            cm_sb[:, ch: ch + 1])
            nc.sync.dma_start(out=phi_in.ap()[:, :, 1: 1 + NCH], in_=phik_m)
            if not solo:
                nc.gpsimd.collective_compute(
                    "AllReduce", mybir.AluOpType.add,
                    replica_groups=[list(range(n_cores))],
                    ins=[phi_in.ap().opt()], outs=[phi_out.ap().opt()],
                )

            # ---- routing scores + top-2 chunk indices ----
            phis = smallp.tile([128, NT, 1 + NCH], F32, tag="phis")
            nc.sync.dma_start(out=phis, in_=(phi_in if solo else phi_out).ap())
            prod = smallp.tile([128, NT, NCH], F32R, tag="prodsc")
            nc.vector.tensor_tensor(
                prod, phis[:, :, 1: 1 + NCH],
                phis[:, :, 0:1].to_broadcast((128, NT, NCH)),
                mybir.AluOpType.mult)
            psc = pa_ss.tile([1, NH * NCH], F32, tag="pss", name="psc")
            nc.tensor.matmul(psc, lhsT=ones_col,
                             rhs=prod[:, :, :].rearrange("p t c -> p (t c)"),
                             start=True, stop=True)
            sc = smallp.tile([1, NH * NCH], F32, tag="sc")
            nc.vector.tensor_copy(out=sc, in_=psc)
            scv = sc[:, :].rearrange("p (h c) -> p h c", c=NCH)
            m1s = smallp.tile([1, NH], F32, tag="m1s")
            nc.vector.reduce_max(out=m1s, in_=scv, axis=mybir.AxisListType.X)
            is1 = smallp.tile([1, NH * NCH], F32, tag="is1")
            nc.vector.tensor_tensor(
                is1[:, :].rearrange("p (h c) -> p h c", c=NCH),
                scv, m1s[:, :, None].to_broadcast((1, NH, NCH)),
                mybir.AluOpType.is_ge)
            nc.vector.tensor_scalar_mul(is1, is1, 1e30)
            masked = smallp.tile([1, NH * NCH], F32, tag="masked")
            nc.vector.tensor_tensor(masked, sc, is1, mybir.AluOpType.subtract)
            m2s = smallp.tile([1, NH], F32, tag="m2s")
            nc.vector.reduce_max(out=m2s,
                                 in_=masked[:, :].rearrange("p (h c) -> p h c", c=NCH),
                                 axis=mybir.AxisListType.X)
            # chunk indices: i1 = argmax, i2 = arg-2nd-max (per head)
            iota4 = smallp.tile([1, NCH], F32, tag="iota4")
            nc.gpsimd.iota(iota4.bitcast(mybir.dt.int32), pattern=[[1, NCH]],
                           base=0, channel_multiplier=0)
            nc.vector.tensor_copy(out=iota4, in_=iota4.bitcast(mybir.dt.int32))
            is2 = smallp.tile([1, NH * NCH], F32, tag="is2")
            nc.vector.tensor_tensor(
                is2[:, :].rearrange("p (h c) -> p h c", c=NCH),
                masked[:, :].rearrange("p (h c) -> p h c", c=NCH),
                m2s[:, :, None].to_broadcast((1, NH, NCH)),
                mybir.AluOpType.is_ge)
            nc.vector.tensor_scalar_mul(is1, is1, 1e-30)  # undo 1e30 scale -> 0/1
            idxf = smallp.tile([1, NH, 2], F32, tag="idxf")
            w1 = smallp.tile([1, NH * NCH], F32, tag="w1")
            nc.vector.tensor_tensor(
                w1[:, :].rearrange("p (h c) -> p h c", c=NCH),
                is1[:, :].rearrange("p (h c) -> p h c", c=NCH),
                iota4[:, None, :].to_broadcast((1, NH, NCH)),
                mybir.AluOpType.mult)
            nc.vector.reduce_sum(out=idxf[:, :, 0], in_=w1[:, :].rearrange(
                "p (h c) -> p h c", c=NCH), axis=mybir.AxisListType.X)
            nc.vector.tensor_tensor(
                w1[:, :].rearrange("p (h c) -> p h c", c=NCH),
                is2[:, :].rearrange("p (h c) -> p h c", c=NCH),
                iota4[:, None, :].to_broadcast((1, NH, NCH)),
                mybir.AluOpType.mult)
            nc.vector.reduce_sum(out=idxf[:, :, 1], in_=w1[:, :].rearrange(
                "p (h c) -> p h c", c=NCH), axis=mybir.AxisListType.X)
            idx_i32 = persist.tile([1, NH * 2], mybir.dt.int32)
            nc.vector.tensor_copy(out=idx_i32,
                                  in_=idxf[:, :, :].rearrange("p h s -> p (h s)"))

        # ---------------- Phase B: attention ----------------
        otp = top.enter_context(tc.tile_pool(name="otp", bufs=1))
        oT_sb = otp.tile([128, NT, TOK], BF16)
        if phases == "a":
            return _finish(nc)
        QS = [(0, 512), (512, TOK - 512)]  # bank-aligned query splits
        wop = top.enter_context(tc.tile_pool(name="wo", bufs=12))
        from concourse.bass import ds as _ds
        with (
            tc.tile_pool(name="kv", bufs=10) as kvp,
            tc.tile_pool(name="ktl", bufs=2) as ktlp,
            tc.tile_pool(name="ebf", bufs=3) as ep,
            tc.tile_pool(name="accp", bufs=6) as accp,
            tc.tile_pool(name="dacc", bufs=2) as dp,
            tc.tile_pool(name="bsm", bufs=2) as bsm,
            tc.tile_pool(name="pb_s", bufs=2, space="PSUM") as pb_s,
            tc.tile_pool(name="pb_o", bufs=4, space="PSUM") as pb_o,
        ):
            # flat software-pipelined stream over all (head, kv-tile) steps:
            # the logits matmul for step i+1 is emitted BEFORE step i's PV so
            # the in-order PE queue never head-blocks on the exp semaphore.
            n_mm = 4 * NKT + 1
            steps = [(h, bi, kt) for h in range(NH)
                     for bi, kt in ([(b, k) for b in range(4)
                                     for k in range(NKT)] + [(4, 0)])]
            st = {}

            def head_setup(h):
                pos = [pb_o.tile([128, 512], F32, tag="po", name=f"po{h}_{qb}")
                       for qb in range(2)]
                blk_regs = []
                for sel in range(2):
                    iv = nc.values_load(
                        idx_i32[0:1, h * 2 + sel: h * 2 + sel + 1],
                        min_val=0, max_val=NCH - 1,
                        skip_runtime_bounds_check=True)
                    blk_regs.append(iv)
                ktail = ktlp.tile([128, NTAIL], BF16, tag="ktail")
                vtail = ktlp.tile([NTAIL, HD], BF16, tag="vtail")
                kts, vbs = [], []
                for bi in range(4):
                    blk = blk_regs[bi // 2] * 2 + (bi % 2)
                    kv_b = (ag_in.ap() if solo
                            else ag_out.ap()[_ds(blk, 1)][0])
                    kT_b = kvp.tile([128, TOK], BF16, tag="kb",
                                    name=f"kT{h}_{bi}")
                    k_view = kv_b[0].rearrange("(t p n) -> t p n",
                                               p=128, n=TOK)[h]
                    nc.sync.dma_start(out=kT_b, in_=k_view)
                    nc.sync.dma_start(
                        out=ktail[:, bi * KTAIL:(bi + 1) * KTAIL],
                        in_=k_view[:, NKT * 128: TOK])
                    V_b = kvp.tile([128, NKT, HD], BF16, tag="vb",
                                   name=f"V{h}_{bi}")
                    nc.sync.dma_start(
                        out=V_b,
                        in_=kv_b[1][0:NKT * 128 * D]
                        .rearrange("(n p d) -> p n d", p=128, d=D)
                        [:, :, h * HD:(h + 1) * HD])
                    nc.sync.dma_start(
                        out=vtail[bi * KTAIL:(bi + 1) * KTAIL, :],
                        in_=kv_b[1].rearrange("(t d) -> t d", d=D)
                        [NKT * 128: TOK, h * HD:(h + 1) * HD])
                    kts.append(kT_b)
                    vbs.append(V_b)
                st[h] = dict(pos=pos, ktail=ktail, vtail=vtail, kts=kts,
                             vbs=vbs, ebs={}, accs=[], ps={}, etail=None)

            def emit_ps(i):
                h, bi, kt = steps[i]
                if (bi, kt) == (0, 0) and h not in st:
                    head_setup(h)
                s = st[h]
                ps = pb_s.tile([128, TOK], F32, tag="ps", name=f"ps{i}")
                if bi == 4:
                    lhs, rows = s["ktail"][:, :], NTAIL
                else:
                    lhs, rows = s["kts"][bi][:, kt * 128:(kt + 1) * 128], 128
                for qb, (q0, qn) in enumerate(QS):
                    nc.tensor.matmul(ps[0:rows, q0:q0 + qn], lhsT=lhs,
                                     rhs=qbf[:, h, q0:q0 + qn],
                                     start=True, stop=True)
                s["ps"][(bi, kt)] = ps

            pending = []

            def emit_midA(h, dr):
                # denominator partition-reduce, first query half only: the
                # two pdp matmuls are split across two drain slots so each
                # displaces the logits stream by half as much
                pdp = pb_s.tile([1, TOK], F32, tag="ps", name=f"pdp{h}")
                q0, qn = QS[0]
                nc.tensor.matmul(pdp[:, q0:q0 + qn], lhsT=ones_col_bf,
                                 rhs=dr[:, q0:q0 + qn], start=True, stop=True)
                return pdp

            def emit_midB(h, dr, pdp):
                q0, qn = QS[1]
                nc.tensor.matmul(pdp[:, q0:q0 + qn], lhsT=ones_col_bf,
                                 rhs=dr[:, q0:q0 + qn], start=True, stop=True)
                rec = bsm.tile([1, TOK], BF16, tag="rec", name=f"rec{h}")
                with nc.allow_low_precision(reason="softmax denom"):
                    nc.vector.reciprocal(out=rec, in_=pdp)
                # DRAM round-trip broadcast: replicate the reciprocal row to
                # 128 partitions on the DMA engines (keeps PE out of the
                # per-head tail entirely)
                nc.sync.dma_start(out=rec_dram.ap()[h:h + 1, :], in_=rec)
                return rec

            def emit_finalize(h, rec, pos):
                rb_sb = bsm.tile([128, TOK], BF16, tag="rbsb")
                nc.sync.dma_start(
                    out=rb_sb,
                    in_=rec_dram.ap()[h:h + 1, :].to_broadcast((128, TOK)))
                for qb, (q0, qn) in enumerate(QS):
                    po_bf = bsm.tile([128, 512], BF16, tag="pobf",
                                     name=f"pobf{h}_{qb}")
                    nc.vector.tensor_copy(out=po_bf[:, :qn],
                                          in_=pos[qb][:, :qn])
                    nc.vector.tensor_tensor(oT_sb[:, h, q0:q0 + qn],
                                            po_bf[:, :qn],
                                            rb_sb[:, q0:q0 + qn],
                                            mybir.AluOpType.mult)

            def drain(i):
                while pending and pending[0][0] <= i:
                    _, kind, fh, fdat, fpos = pending.pop(0)
                    if kind == "midA":
                        pdp = emit_midA(fh, fdat)
                        pending.append((i + 1, "midB", fh, (fdat, pdp), fpos))
                        pending.sort(key=lambda e: e[0])
                    elif kind == "midB":
                        rec = emit_midB(fh, fdat[0], fdat[1])
                        pending.append((i + 3, "fin", fh, rec, fpos))
                        pending.sort(key=lambda e: e[0])
                    else:
                        emit_finalize(fh, fdat, fpos)

            emit_ps(0)
            emit_ps(1)
            wo_tiles = []
            for i, (h, bi, kt) in enumerate(steps):
                if i + 2 < len(steps):
                    emit_ps(i + 2)
                # prefetch the NEXT head's kv DMAs well ahead of its first
                # logits so the boundary exp never waits on the gather
                j = i + 6
                if j < len(steps) and steps[j][1:] == (0, 0) \
                        and steps[j][0] not in st:
                    head_setup(steps[j][0])
                drain(i)
                if i == 30:
                    # wo loads issued mid-attention: off the boundary-critical
                    # DMA queue, done long before the output projection
                    for k in range(NT):
                        wt = wop.tile([128, D], BF16, tag="wo", name=f"wo{k}")
                        nc.sync.dma_start(out=wt, in_=woT.ap()[k])
                        wo_tiles.append(wt)
                s = st[h]
                ps = s["ps"].pop((bi, kt))
                if bi == 4:
                    etail = bsm.tile([NTAIL, TOK], BF16, tag="etail")
                    nc.scalar.activation(out=etail, in_=ps[0:NTAIL, :],
                                         func=mybir.ActivationFunctionType.Exp,
                                         scale=SM_SCALE)
                    s["etail"] = etail
                    e_ap, lhsT_pv, i_mm = etail[:, :], s["vtail"][:, :], 4 * NKT
                else:
                    if kt == 0:
                        s["ebs"][bi] = ep.tile([128, NKT, TOK], BF16, tag="eb",
                                               name=f"eb{h}_{bi}")
                    eb = s["ebs"][bi]
                    nc.scalar.activation(out=eb[:, kt, :], in_=ps,
                                         func=mybir.ActivationFunctionType.Exp,
                                         scale=SM_SCALE)
                    e_ap, lhsT_pv = eb[:, kt, :], s["vbs"][bi][:, kt, :]
                    i_mm = bi * NKT + kt
                for qb, (q0, qn) in enumerate(QS):
                    nc.tensor.matmul(s["pos"][qb][:, :qn], lhsT=lhsT_pv,
                                     rhs=e_ap[:, q0:q0 + qn],
                                     start=(i_mm == 0), stop=(i_mm == n_mm - 1))
                if bi < 4 and kt == NKT - 1:
                    # per-block pairwise denominator tree (bf16, 2x DVE)
                    eb = s["ebs"][bi]
                    t1 = accp.tile([128, TOK], BF16, tag="dt", name="t1")
                    nc.vector.tensor_tensor(t1, eb[:, 0, :], eb[:, 1, :],
                                            mybir.AluOpType.add)
                    t2 = accp.tile([128, TOK], BF16, tag="dt", name="t2")
                    nc.vector.tensor_tensor(t2, eb[:, 2, :], eb[:, 3, :],
                                            mybir.AluOpType.add)
                    t3 = accp.tile([128, TOK], BF16, tag="dt", name="t3")
                    nc.vector.tensor_tensor(t3, eb[:, 4, :], eb[:, 5, :],
                                            mybir.AluOpType.add)
                    nc.vector.tensor_tensor(t1, t1, t2, mybir.AluOpType.add)
                    acc = accp.tile([128, TOK], BF16, tag="acc",
                                    name=f"acc{h}_{bi}")
                    nc.vector.tensor_tensor(acc, t1, t3, mybir.AluOpType.add)
                    s["accs"].append(acc)
                if bi == 4:
                    # den = (acc0+acc1+tail) + (acc2+acc3); off the PE queue
                    accs, etail = s["accs"], s["etail"]
                    s01 = accp.tile([128, TOK], BF16, tag="dt", name="s01")
                    nc.vector.tensor_tensor(s01, accs[0], accs[1],
                                            mybir.AluOpType.add)
                    nc.vector.tensor_tensor(s01[0:NTAIL, :], s01[0:NTAIL, :],
                                            etail, mybir.AluOpType.add)
                    s23 = accp.tile([128, TOK], BF16, tag="dt", name="s23")
                    nc.vector.tensor_tensor(s23, accs[2], accs[3],
                                            mybir.AluOpType.add)
                    dr = dp.tile([128, TOK], BF16, tag="dr", name=f"dr{h}")
                    nc.vector.tensor_tensor(dr, s01, s23, mybir.AluOpType.add)
                    pending.append((i + 5, "midA", h, dr, s["pos"]))
                    pending.sort(key=lambda e: e[0])
                    del st[h]
            while pending:
                _, kind, fh, fdat, fpos = pending.pop(0)
                if kind == "midA":
                    pdp = emit_midA(fh, fdat)
                    rec = emit_midB(fh, fdat, pdp)
                    emit_finalize(fh, rec, fpos)
                elif kind == "midB":
                    rec = emit_midB(fh, fdat[0], fdat[1])
                    emit_finalize(fh, rec, fpos)
                else:
                    emit_finalize(fh, fdat, fpos)

        # ---------------- out projection ----------------
        if phases == "ab":
            return _finish(nc)
        with (
            tc.tile_pool(name="osb", bufs=4) as osb,
            tc.tile_pool(name="po_mm", bufs=4, space="PSUM") as po_mm,
        ):
            groups = [(tb, nb) for tb in range(7) for nb in range(3)]

            def og_mm(pO, tb, nb, m, ks):
                for k in ks:
                    nc.tensor.matmul(pO[:m, :],
                                     lhsT=oT_sb[:, k, tb * 128: tb * 128 + m],
                                     rhs=wo_tiles[k][:, nb * 512:(nb + 1) * 512],
                                     start=(k == 0), stop=(k == NT - 1))

            def og_out(pO, tb, nb, m):
                ob = osb.tile([128, 512], F32, tag="ob")
                nc.scalar.copy(out=ob[:m, :], in_=pO[:m, :])
                nc.sync.dma_start(
                    out=out.ap()[tb * 128: tb * 128 + m,
                                 nb * 512:(nb + 1) * 512],
                    in_=ob[:m, :])

            # first three groups interleave: their head-11 matmuls come after
            # 33 ready matmuls so PE doesn't head-block on the last head's
            # deferred finalize writing oT[:, 11, :]
            pre = []
            for gi in range(3):
                pg_ = po_mm.tile([128, 512], F32, tag="pO", name=f"pOi{gi}")
                og_mm(pg_, 0, gi, 128, range(NT - 1))
                pre.append(pg_)
            for gi in range(3):
                og_mm(pre[gi], 0, gi, 128, [NT - 1])
                og_out(pre[gi], 0, gi, 128)
            for tb, nb in groups[3:]:
                m = 128 if tb < 6 else MTAIL
                pO = po_mm.tile([128, 512], F32, tag="pO")
                og_mm(pO, tb, nb, m, range(NT))
                og_out(pO, tb, nb, m)

    return _finish(nc)


def _finish(nc):
    nc.compile()
    return nc


# ---------------- host-side prep ----------------

def _perm():
    p = np.arange(D).reshape(NH, C, 2)
    return np.concatenate([p[:, :, 0], p[:, :, 1]], axis=1).reshape(-1)


def make_fcis(freqs, grid_sizes):
    f, h, w = [int(v) for v in np.asarray(grid_sizes)[0]]
    c1 = C - 2 * (C // 3)
    c2 = C // 3
    fq = np.asarray(freqs, np.float32)
    ff = np.broadcast_to(fq[:f, None, None, :c1], (f, h, w, c1, 2))
    fh = np.broadcast_to(fq[None, :h, None, c1:c1 + c2], (f, h, w, c2, 2))
    fw = np.broadcast_to(fq[None, None, :w, c1 + c2:c1 + 2 * c2], (f, h, w, c2, 2))
    fcis = np.concatenate([ff, fh, fw], axis=3).reshape(f * h * w, C, 2)
    if fcis.shape[0] == 1 and S > 1:
        fcis = np.broadcast_to(fcis, (S, C, 2))
    return fcis


def host_prep(inputs):
    """inputs: the full reference input dict -> per-core in_maps."""
    import ml_dtypes
    bf16 = ml_dtypes.bfloat16
    x = np.asarray(inputs["x"], np.float32)
    freqs = np.asarray(inputs["freqs"], np.float32)
    grid_sizes = np.asarray(inputs["grid_sizes"])
    assert x.shape == (1, S, D)
    assert int(np.asarray(inputs["chunk_size"])) == S // NCH
    assert int(np.asarray(inputs["top_k"])) == 2

    perm = _perm()
    wq = np.asarray(inputs["wq"], np.float32)[perm]
    wk = np.asarray(inputs["wk"], np.float32)[perm]
    wv = np.asarray(inputs["wv"], np.float32)
    wo = np.asarray(inputs["wo"], np.float32)
    for b in ("bq", "bk", "bv", "bo"):
        assert not np.any(np.asarray(inputs[b])), f"nonzero bias {b} unsupported"
    for g in ("gq", "gk"):
        assert np.all(np.asarray(inputs[g]) == 1.0), f"non-unit gain {g} unsupported"

    xT = np.ascontiguousarray(x[0].T).reshape(NT, 128, S).astype(bf16)
    wqT = np.ascontiguousarray(wq.T).reshape(NT, 128, D).astype(bf16)
    wkT = np.ascontiguousarray(wk.T).reshape(NT, 128, D).astype(bf16)
    wvT = np.ascontiguousarray(wv.T).reshape(NT, 128, D).astype(bf16)
    woT = np.ascontiguousarray(wo.T).reshape(NT, 128, D).astype(bf16)

    fcis = make_fcis(freqs, grid_sizes)  # [S, C, 2]
    frT = fcis[:, :, 0].T  # [C, S]
    fiT = fcis[:, :, 1].T
    f11 = np.concatenate([frT, frT], axis=0)  # [128, S]
    f2n = np.concatenate([-fiT, fiT], axis=0)

    in_maps = []
    for c in range(N_CORES):
        sl = slice(c * TOK, (c + 1) * TOK)
        cm = np.zeros((128, NCH), np.float32)
        cm[:, (c * TOK) // (S // NCH)] = 1.0
        in_maps.append({
            "xT": np.ascontiguousarray(xT[:, :, sl]),
            "wqT": wqT, "wkT": wkT, "wvT": wvT, "woT": woT,
            "f11": np.ascontiguousarray(f11[:, sl]).astype(bf16),
            "f2n": np.ascontiguousarray(f2n[:, sl]).astype(bf16),
            "chmask": cm,
        })
    return in_maps


def assemble_out(results):
    return np.concatenate([r["out"] for r in results], axis=0)[None]


# ---------------- harness entry point ----------------

_CACHE = {}


def kernel(**inputs):
    import numpy as _np
    key = "nc"
    if key not in _CACHE:
        _CACHE[key] = build_kernel()
    nc = _CACHE[key]
    in_maps = host_prep(inputs)
    from concourse import bass_utils
    res = bass_utils.run_bass_kernel_spmd(
        nc, in_maps, core_ids=list(range(N_CORES)), trace=False)
    return assemble_out(res.results).astype(_np.float32)


# revision 80
# speedup vs baseline: 1.0053x; 1.0028x over previous
"""nn_CausalWanSelfAttention Trainium2 kernel (8-core SPMD, single launch).

Entry point: kernel(**inputs) -> np.ndarray [1, 6240, 1536] float32.

Strategy (v2 — bf16 datapath, packed kv, software-pipelined attention):
  - Phase A, token-sharded (780 tokens/core): q/k/v projections as bf16
    matmuls (f32 PSUM accumulation), rmsnorm via bf16 square + ones-matmul
    partition reduction. 3D-RoPE on pair-de-interleaved channels: the
    partition-half swap runs on the DMA engines (SBUF->SBUF), then
    dst = raw*[fr;fr] + swap(raw)*[-fi;fi] as same-partition bf16 ops
    (2x DVE mode); the final add fuses the phi token-reduction via
    scalar_tensor_tensor(accum_out=...). The per-token rmsnorm scale
    folds into the rope tables once per projection.
  - One AllGather ships (k^T, v) in bf16 (780 tokens per block, no pad);
    one small AllReduce combines the routing means phi_q/phi_k.
  - Top-2-of-4 chunk routing on device; per-head chunk indices drive
    dynamically-addressed DMA gathers of the selected K/V blocks.
  - Phase B, query-sharded, emitted as ONE flat software-pipelined stream
    over all (head, kv-tile) steps: the logits matmul for step i+1 is
    emitted before step i's PV so the in-order PE queue never head-blocks
    on the exp semaphore, and each head's denominator reduce / reciprocal
    / o^T normalization are deferred several steps into the next head.
    24 full 128-row kv tiles plus ONE packed tail tile assembled from the
    4 selected blocks' 12 leftover rows (25 tile-ops per engine, not 28).
    exp on the scalar engine (the pacing engine, ~87% busy); softmax
    denominator via a bf16 pairwise tree on the vector engine; PV
    accumulation in PSUM; the row-parallel output projection reads o^T
    (bf16) from SBUF.
"""

from contextlib import ExitStack

import numpy as np

import concourse.bacc as bacc
import concourse.mybir as mybir
import concourse.tile as tile

F32R = mybir.dt.float32r
F32 = mybir.dt.float32
BF16 = mybir.dt.bfloat16
KV_BYTES_PER_ELEM = 2    # K/V ship over the AllGather in bf16

N_CORES = 8
S, D, NH, HD, C = 6240, 1536, 12, 128, 64
NT = D // 128          # 12 channel tiles
TOK = S // N_CORES     # 780 tokens per core
NBLK = N_CORES         # 8 kv blocks (one per core)
NCH = 4                # routing chunks
NKT = 6                # full 128-row kv tiles per block (768 rows)
KTAIL = TOK - NKT * 128   # 12 leftover kv rows per block
NTAIL = 4 * KTAIL         # 48 packed tail rows (4 selected blocks)
EPS = 1e-6
SM_SCALE = 1.0 / float(np.sqrt(HD))
KV_ELEMS = NT * 128 * TOK  # bf16 elements per (k or v) per core
MTAIL = TOK - 6 * 128  # 12 (token tail within a core block)


def build_kernel(n_cores=N_CORES, solo=False, phases="full"):
    nc = bacc.Bacc("TRN2", target_bir_lowering=False, debug=False,
                   num_devices=n_cores)

    xT = nc.dram_tensor("xT", [NT, 128, TOK], BF16, kind="ExternalInput")
    wqT = nc.dram_tensor("wqT", [NT, 128, D], BF16, kind="ExternalInput")
    wkT = nc.dram_tensor("wkT", [NT, 128, D], BF16, kind="ExternalInput")
    wvT = nc.dram_tensor("wvT", [NT, 128, D], BF16, kind="ExternalInput")
    woT = nc.dram_tensor("woT", [NT, 128, D], BF16, kind="ExternalInput")
    f11 = nc.dram_tensor("f11", [128, TOK], BF16, kind="ExternalInput")
    f2n = nc.dram_tensor("f2n", [128, TOK], BF16, kind="ExternalInput")
    chmask = nc.dram_tensor("chmask", [128, NCH], F32, kind="ExternalInput")

    out = nc.dram_tensor("out", [TOK, D], F32, kind="ExternalOutput")

    # collective buffers
    ag_in = nc.dram_tensor("ag_in", [2, KV_ELEMS], BF16)
    ag_out = nc.dram_tensor("ag_out", [NBLK, 2, KV_ELEMS], BF16,
                            addr_space="Shared")
    phi_in = nc.dram_tensor("phi_in", [128, NT, 1 + NCH], F32)
    phi_out = nc.dram_tensor("phi_out", [128, NT, 1 + NCH], F32,
                             addr_space="Shared")
    rec_dram = nc.dram_tensor("rec_dram", [NH, TOK], BF16)

    k_in_view = ag_in.ap()[0].rearrange("(t p n) -> t p n", p=128, n=TOK)
    v_in_view = ag_in.ap()[1].rearrange("(k d) -> k d", d=D)

    ones_col_t = nc.inline_tensor(np.ones((128, 1), np.float32), name="ones_col")
    ones_row_t = nc.inline_tensor(np.ones((1, 128), np.float32), name="ones_row")

    with tile.TileContext(nc) as tc, ExitStack() as top:
        consts = top.enter_context(tc.tile_pool(name="consts", bufs=1))
        ones_col = consts.tile([128, 1], F32R)
        nc.sync.dma_start(out=ones_col, in_=ones_col_t.ap().bitcast(F32R))
        ones_row = consts.tile([1, 128], F32R)
        nc.sync.dma_start(out=ones_row, in_=ones_row_t.ap().bitcast(F32R))
        ones_col_bf = consts.tile([128, 1], BF16)
        nc.vector.memset(ones_col_bf, 1.0)
        f11_sb = consts.tile([128, TOK], BF16)
        nc.sync.dma_start(out=f11_sb, in_=f11[:, :])
        f2n_sb = consts.tile([128, TOK], BF16)
        nc.sync.dma_start(out=f2n_sb, in_=f2n[:, :])
        cm_sb = consts.tile([128, NCH], F32)
        nc.sync.dma_start(out=cm_sb, in_=chmask[:, :])
        eps_sb = consts.tile([1, 1], F32)
        nc.vector.memset(eps_sb, EPS)
        # preload the Exp activation table so the first attention exp
        # doesn't pay the 1.3us LoadActFuncSet at the phase boundary
        warm = consts.tile([1, 1], F32)
        nc.scalar.activation(out=warm, in_=eps_sb,
                             func=mybir.ActivationFunctionType.Exp)

        # persistent across phases
        persist = top.enter_context(tc.tile_pool(name="persist", bufs=1))
        qbf = persist.tile([128, NT, TOK], BF16)
        phiq_sb = persist.tile([128, NT], F32)
        phik_sb = persist.tile([128, NT], F32)

        # ---------------- Phase A ----------------
        with (
            tc.tile_pool(name="xin", bufs=1) as xin,
            tc.tile_pool(name="wts", bufs=24) as wts,
            tc.tile_pool(name="pa_mm", bufs=2, space="PSUM") as pa_mm,
            tc.tile_pool(name="pa_ss", bufs=2, space="PSUM") as pa_ss,
            tc.tile_pool(name="pa_g", bufs=1, space="PSUM") as pa_g,
            tc.tile_pool(name="raw", bufs=1) as rawp,
            tc.tile_pool(name="sqp", bufs=3) as sqp,
            tc.tile_pool(name="rope", bufs=3) as ropep,
            tc.tile_pool(name="outbf", bufs=3) as outbf,
            tc.tile_pool(name="small", bufs=2) as smallp,
            tc.tile_pool(name="frqp", bufs=2) as frqp,
        ):
            xT_sb = xin.tile([128, NT, TOK], BF16)

            def v_proj():
                # natural layout [tok, ch]; emitted between k and q proj so
                # its matmuls overlap the k rope tail on DVE. The first weight
                # block's loads interleave with the xT loads so matmul k can
                # start as soon as (xT[k], wv0[k]) land.
                for nb in range(4):
                    wv_nb = []
                    for k in range(NT):
                        if nb == 0:
                            nc.sync.dma_start(out=xT_sb[:, k, :], in_=xT.ap()[k])
                        wt = wts.tile([128, 384], BF16, tag="wt",
                                      name=f"wv{nb}_{k}")
                        nc.sync.dma_start(
                            out=wt, in_=wvT.ap()[k, :, nb * 384:(nb + 1) * 384])
                        wv_nb.append(wt)
                    for tb in range(7):
                        m = 128 if tb < 6 else MTAIL
                        pv = pa_mm.tile([128, 384], F32, tag="pmm",
                                        name=f"pv{nb}_{tb}")
                        for k in range(NT):
                            nc.tensor.matmul(
                                pv[:m, :],
                                lhsT=xT_sb[:, k, tb * 128: tb * 128 + m],
                                rhs=wv_nb[k],
                                start=(k == 0), stop=(k == NT - 1),
                            )
                        vbf = outbf.tile([128, 384], BF16, tag="vbf")
                        nc.scalar.copy(out=vbf[:m, :], in_=pv[:m, :])
                        nc.sync.dma_start(
                            out=v_in_view[tb * 128: tb * 128 + m,
                                          nb * 384:(nb + 1) * 384],
                            in_=vbf[:m, :],
                        )

            # ---- q/k projections (transposed layout [ch, tok]) ----
            QSA = [(0, 512), (512, TOK - 512)]

            def qk_proj(wdram, is_q):
                tagq = "q" if is_q else "k"
                raw = rawp.tile([128, NT, TOK], BF16, tag=f"raw{tagq}")
                psss = [pa_ss.tile([1, 512], F32, tag="pss", name=f"pss{tagq}{i}")
                        for i in range(2)]
                for half in range(4):
                    w_half = []
                    for k in range(NT):
                        wt = wts.tile([128, 384], BF16, tag="wt",
                                      name=f"w{tagq}{half}_{k}")
                        nc.sync.dma_start(
                            out=wt, in_=wdram.ap()[k, :, half * 384:(half + 1) * 384])
                        w_half.append(wt)
                    for oi in range(3):
                        ot = half * 3 + oi
                        pk = pa_mm.tile([128, TOK], F32, tag="pmm")
                        for qi, (q0, qn) in enumerate(QSA):
                            for k in range(NT):
                                nc.tensor.matmul(
                                    pk[:, q0:q0 + qn],
                                    lhsT=w_half[k][:, oi * 128:(oi + 1) * 128],
                                    rhs=xT_sb[:, k, q0:q0 + qn],
                                    start=(k == 0), stop=(k == NT - 1),
                                )
                        nc.scalar.copy(out=raw[:, ot, :], in_=pk)
                        sq = sqp.tile([128, TOK], BF16, tag="sq")
                        nc.vector.tensor_tensor(sq, raw[:, ot, :], raw[:, ot, :],
                                                mybir.AluOpType.mult)
                        for qi, (q0, qn) in enumerate(QSA):
                            nc.tensor.matmul(psss[qi][:, :qn], lhsT=ones_col_bf,
                                             rhs=sq[:, q0:q0 + qn],
                                             start=(ot == 0), stop=(ot == NT - 1))
                rs = smallp.tile([1, TOK], F32R, tag="rs")
                for qi, (q0, qn) in enumerate(QSA):
                    rs1 = smallp.tile([1, 512], F32, tag="rs1")
                    nc.scalar.activation(out=rs1[:, :qn], in_=psss[qi][:, :qn],
                                         func=mybir.ActivationFunctionType.Sqrt,
                                         bias=eps_sb[0:1, 0:1], scale=1.0 / D)
                    with nc.allow_low_precision(reason="rms scale in f32r"):
                        nc.vector.reciprocal(out=rs[:, q0:q0 + qn], in_=rs1[:, :qn])
                # broadcast rs to 128 partitions, fold into the rope tables
                prs = pa_g.tile([128, TOK], F32, tag="pg", name=f"prs{tagq}")
                for qi, (q0, qn) in enumerate(QSA):
                    nc.tensor.matmul(prs[:, q0:q0 + qn], lhsT=ones_row,
                                     rhs=rs[:, q0:q0 + qn], start=True, stop=True)
                prs_bf = smallp.tile([128, TOK], BF16, tag="prsbf")
                nc.scalar.copy(out=prs_bf, in_=prs)
                G1 = frqp.tile([128, TOK], BF16, tag="G1")
                nc.vector.tensor_tensor(G1, f11_sb, prs_bf, mybir.AluOpType.mult)
                G2 = frqp.tile([128, TOK], BF16, tag="G2")
                nc.vector.tensor_tensor(G2, f2n_sb, prs_bf, mybir.AluOpType.mult)
                for ot in range(NT):
                    # rope: pairs de-interleaved -> a=rows 0:C, b=rows C:128;
                    # dst = raw*[fr;fr] + swap(raw)*[-fi;fi], swap via DMA
                    rsw = ropep.tile([128, TOK], BF16, tag="rsw", bufs=12)
                    nc.sync.dma_start(out=rsw[0:C, :], in_=raw[C:128, ot, :])
                    nc.sync.dma_start(out=rsw[C:128, :], in_=raw[0:C, ot, :])
                    m1 = ropep.tile([128, TOK], BF16, tag="m1")
                    nc.vector.tensor_tensor(m1, raw[:, ot, :], G1,
                                            mybir.AluOpType.mult)
                    m2 = ropep.tile([128, TOK], BF16, tag="m2")
                    nc.vector.tensor_tensor(m2, rsw, G2,
                                            mybir.AluOpType.mult)
                    if is_q:
                        dst = qbf[:, ot, :]
                    else:
                        kbf = outbf.tile([128, TOK], BF16, tag="kbf")
                        dst = kbf[:, :]
                    phi_dst = phiq_sb if is_q else phik_sb
                    nc.vector.scalar_tensor_tensor(
                        out=dst, in0=m1, scalar=1.0, in1=m2,
                        op0=mybir.AluOpType.mult, op1=mybir.AluOpType.add,
                        accum_out=phi_dst[:, ot: ot + 1])
                    if not is_q:
                        nc.sync.dma_start(out=k_in_view[ot, :, 0:TOK], in_=kbf)

            v_proj()
            qk_proj(wkT, is_q=False)

            # AllGather (kT, v) once k and v blocks are written
            if not solo:
                nc.gpsimd.collective_compute(
                    "AllGather", mybir.AluOpType.bypass,
                    replica_groups=[list(range(n_cores))],
                    ins=[ag_in.ap().opt()], outs=[ag_out.ap().opt()],
                )

            qk_proj(wqT, is_q=True)

            # ---- phi AllReduce ----
            nc.sync.dma_start(out=phi_in.ap()[:, :, 0:1],
                              in_=phiq_sb[:, :, None])
            phik_m = smallp.tile([128, NT, NCH], F32, tag="phikm")
            for ch in range(NCH):
                nc.vector.tensor_scalar_mul(phik_m[:, :, ch], phik_sb,
                                            cm_sb[:, ch: ch + 1])
            nc.sync.dma_start(out=phi_in.ap()[:, :, 1: 1 + NCH], in_=phik_m)
            if not solo:
                nc.gpsimd.collective_compute(
                    "AllReduce", mybir.AluOpType.add,
                    replica_groups=[list(range(n_cores))],
                    ins=[phi_in.ap().opt()], outs=[phi_out.ap().opt()],
                )

            # ---- routing scores + top-2 chunk indices ----
            phis = smallp.tile([128, NT, 1 + NCH], F32, tag="phis")
            nc.sync.dma_start(out=phis, in_=(phi_in if solo else phi_out).ap())
            prod = smallp.tile([128, NT, NCH], F32R, tag="prodsc")
            nc.vector.tensor_tensor(
                prod, phis[:, :, 1: 1 + NCH],
                phis[:, :, 0:1].to_broadcast((128, NT, NCH)),
                mybir.AluOpType.mult)
            psc = pa_ss.tile([1, NH * NCH], F32, tag="pss", name="psc")
            nc.tensor.matmul(psc, lhsT=ones_col,
                             rhs=prod[:, :, :].rearrange("p t c -> p (t c)"),
                             start=True, stop=True)
            sc = smallp.tile([1, NH * NCH], F32, tag="sc")
            nc.vector.tensor_copy(out=sc, in_=psc)
            scv = sc[:, :].rearrange("p (h c) -> p h c", c=NCH)
            m1s = smallp.tile([1, NH], F32, tag="m1s")
            nc.vector.reduce_max(out=m1s, in_=scv, axis=mybir.AxisListType.X)
            is1 = smallp.tile([1, NH * NCH], F32, tag="is1")
            nc.vector.tensor_tensor(
                is1[:, :].rearrange("p (h c) -> p h c", c=NCH),
                scv, m1s[:, :, None].to_broadcast((1, NH, NCH)),
                mybir.AluOpType.is_ge)
            nc.vector.tensor_scalar_mul(is1, is1, 1e30)
            masked = smallp.tile([1, NH * NCH], F32, tag="masked")
            nc.vector.tensor_tensor(masked, sc, is1, mybir.AluOpType.subtract)
            m2s = smallp.tile([1, NH], F32, tag="m2s")
            nc.vector.reduce_max(out=m2s,
                                 in_=masked[:, :].rearrange("p (h c) -> p h c", c=NCH),
                                 axis=mybir.AxisListType.X)
            # chunk indices: i1 = argmax, i2 = arg-2nd-max (per head)
            iota4 = smallp.tile([1, NCH], F32, tag="iota4")
            nc.gpsimd.iota(iota4.bitcast(mybir.dt.int32), pattern=[[1, NCH]],
                           base=0, channel_multiplier=0)
            nc.vector.tensor_copy(out=iota4, in_=iota4.bitcast(mybir.dt.int32))
            is2 = smallp.tile([1, NH * NCH], F32, tag="is2")
            nc.vector.tensor_tensor(
                is2[:, :].rearrange("p (h c) -> p h c", c=NCH),
                masked[:, :].rearrange("p (h c) -> p h c", c=NCH),
                m2s[:, :, None].to_broadcast((1, NH, NCH)),
                mybir.AluOpType.is_ge)
            nc.vector.tensor_scalar_mul(is1, is1, 1e-30)  # undo 1e30 scale -> 0/1
            idxf = smallp.tile([1, NH, 2], F32, tag="idxf")
            w1 = smallp.tile([1, NH * NCH], F32, tag="w1")
            nc.vector.tensor_tensor(
                w1[:, :].rearrange("p (h c) -> p h c", c=NCH),
                is1[:, :].rearrange("p (h c) -> p h c", c=NCH),
                iota4[:, None, :].to_broadcast((1, NH, NCH)),
                mybir.AluOpType.mult)
            nc.vector.reduce_sum(out=idxf[:, :, 0], in_=w1[:, :].rearrange(
                "p (h c) -> p h c", c=NCH), axis=mybir.AxisListType.X)
            nc.vector.tensor_tensor(
                w1[:, :].rearrange("p (h c) -> p h c", c=NCH),
                is2[:, :].rearrange("p (h c) -> p h c", c=NCH),
                iota4[:, None, :].to_broadcast((1, NH, NCH)),
                mybir.AluOpType.mult)
            nc.vector.reduce_sum(out=idxf[:, :, 1], in_=w1[:, :].rearrange(
                "p (h c) -> p h c", c=NCH), axis=mybir.AxisListType.X)
            idx_i32 = persist.tile([1, NH * 2], mybir.dt.int32)
            nc.vector.tensor_copy(out=idx_i32,
                                  in_=idxf[:, :, :].rearrange("p h s -> p (h s)"))

        # ---------------- Phase B: attention ----------------
        otp = top.enter_context(tc.tile_pool(name="otp", bufs=1))
        oT_sb = otp.tile([128, NT, TOK], BF16)
        if phases == "a":
            return _finish(nc)
        QS = [(0, 512), (512, TOK - 512)]  # bank-aligned query splits
        wop = top.enter_context(tc.tile_pool(name="wo", bufs=12))
        from concourse.bass import ds as _ds
        with (
            tc.tile_pool(name="kv", bufs=10) as kvp,
            tc.tile_pool(name="ktl", bufs=2) as ktlp,
            tc.tile_pool(name="ebf", bufs=3) as ep,
            tc.tile_pool(name="accp", bufs=6) as accp,
            tc.tile_pool(name="dacc", bufs=2) as dp,
            tc.tile_pool(name="bsm", bufs=2) as bsm,
            tc.tile_pool(name="pb_s", bufs=2, space="PSUM") as pb_s,
            tc.tile_pool(name="pb_o", bufs=4, space="PSUM") as pb_o,
        ):
            # flat software-pipelined stream over all (head, kv-tile) steps:
            # the logits matmul for step i+1 is emitted BEFORE step i's PV so
            # the in-order PE queue never head-blocks on the exp semaphore.
            n_mm = 4 * NKT + 1
            steps = [(h, bi, kt) for h in range(NH)
                     for bi, kt in ([(b, k) for b in range(4)
                                     for k in range(NKT)] + [(4, 0)])]
            st = {}

            def head_setup(h):
                pos = [pb_o.tile([128, 512], F32, tag="po", name=f"po{h}_{qb}")
                       for qb in range(2)]
                blk_regs = []
                for sel in range(2):
                    iv = nc.values_load(
                        idx_i32[0:1, h * 2 + sel: h * 2 + sel + 1],
                        min_val=0, max_val=NCH - 1,
                        skip_runtime_bounds_check=True)
                    blk_regs.append(iv)
                ktail = ktlp.tile([128, NTAIL], BF16, tag="ktail")
                vtail = ktlp.tile([NTAIL, HD], BF16, tag="vtail")
                kts, vbs = [], []
                for bi in range(4):
                    blk = blk_regs[bi // 2] * 2 + (bi % 2)
                    kv_b = (ag_in.ap() if solo
                            else ag_out.ap()[_ds(blk, 1)][0])
                    kT_b = kvp.tile([128, TOK], BF16, tag="kb",
                                    name=f"kT{h}_{bi}")
                    k_view = kv_b[0].rearrange("(t p n) -> t p n",
                                               p=128, n=TOK)[h]
                    nc.sync.dma_start(out=kT_b, in_=k_view)
                    nc.sync.dma_start(
                        out=ktail[:, bi * KTAIL:(bi + 1) * KTAIL],
                        in_=k_view[:, NKT * 128: TOK])
                    V_b = kvp.tile([128, NKT, HD], BF16, tag="vb",
                                   name=f"V{h}_{bi}")
                    nc.sync.dma_start(
                        out=V_b,
                        in_=kv_b[1][0:NKT * 128 * D]
                        .rearrange("(n p d) -> p n d", p=128, d=D)
                        [:, :, h * HD:(h + 1) * HD])
                    nc.sync.dma_start(
                        out=vtail[bi * KTAIL:(bi + 1) * KTAIL, :],
                        in_=kv_b[1].rearrange("(t d) -> t d", d=D)
                        [NKT * 128: TOK, h * HD:(h + 1) * HD])
                    kts.append(kT_b)
                    vbs.append(V_b)
                st[h] = dict(pos=pos, ktail=ktail, vtail=vtail, kts=kts,
                             vbs=vbs, ebs={}, accs=[], ps={}, etail=None)

            def emit_ps(i):
                h, bi, kt = steps[i]
                if (bi, kt) == (0, 0) and h not in st:
                    head_setup(h)
                s = st[h]
                ps = pb_s.tile([128, TOK], F32, tag="ps", name=f"ps{i}")
                if bi == 4:
                    lhs, rows = s["ktail"][:, :], NTAIL
                else:
                    lhs, rows = s["kts"][bi][:, kt * 128:(kt + 1) * 128], 128
                for qb, (q0, qn) in enumerate(QS):
                    nc.tensor.matmul(ps[0:rows, q0:q0 + qn], lhsT=lhs,
                                     rhs=qbf[:, h, q0:q0 + qn],
                                     start=True, stop=True)
                s["ps"][(bi, kt)] = ps

            pending = []

            def emit_midA(h, dr):
                # denominator partition-reduce, first query half only: the
                # two pdp matmuls are split across two drain slots so each
                # displaces the logits stream by half as much
                pdp = pb_s.tile([1, TOK], F32, tag="ps", name=f"pdp{h}")
                q0, qn = QS[0]
                nc.tensor.matmul(pdp[:, q0:q0 + qn], lhsT=ones_col_bf,
                                 rhs=dr[:, q0:q0 + qn], start=True, stop=True)
                return pdp

            def emit_midB(h, dr, pdp):
                q0, qn = QS[1]
                nc.tensor.matmul(pdp[:, q0:q0 + qn], lhsT=ones_col_bf,
                                 rhs=dr[:, q0:q0 + qn], start=True, stop=True)
                rec = bsm.tile([1, TOK], BF16, tag="rec", name=f"rec{h}")
                with nc.allow_low_precision(reason="softmax denom"):
                    nc.vector.reciprocal(out=rec, in_=pdp)
                # DRAM round-trip broadcast: replicate the reciprocal row to
                # 128 partitions on the DMA engines (keeps PE out of the
                # per-head tail entirely)
                nc.sync.dma_start(out=rec_dram.ap()[h:h + 1, :], in_=rec)
                return rec

            def emit_finalize(h, rec, pos):
                rb_sb = bsm.tile([128, TOK], BF16, tag="rbsb")
                nc.sync.dma_start(
                    out=rb_sb,
                    in_=rec_dram.ap()[h:h + 1, :].to_broadcast((128, TOK)))
                for qb, (q0, qn) in enumerate(QS):
                    po_bf = bsm.tile([128, 512], BF16, tag="pobf",
                                     name=f"pobf{h}_{qb}")
                    nc.vector.tensor_copy(out=po_bf[:, :qn],
                                          in_=pos[qb][:, :qn])
                    nc.vector.tensor_tensor(oT_sb[:, h, q0:q0 + qn],
                                            po_bf[:, :qn],
                                            rb_sb[:, q0:q0 + qn],
                                            mybir.AluOpType.mult)

            def drain(i):
                while pending and pending[0][0] <= i:
                    _, kind, fh, fdat, fpos = pending.pop(0)
                    if kind == "midA":
                        pdp = emit_midA(fh, fdat)
                        pending.append((i + 1, "midB", fh, (fdat, pdp), fpos))
                        pending.sort(key=lambda e: e[0])
                    elif kind == "midB":
                        rec = emit_midB(fh, fdat[0], fdat[1])
                        pending.append((i + 3, "fin", fh, rec, fpos))
                        pending.sort(key=lambda e: e[0])
                    else:
                        emit_finalize(fh, fdat, fpos)

            emit_ps(0)
            emit_ps(1)
            wo_tiles = []
            groups = [(tb, nb) for tb in range(7) for nb in range(3)]
            early = {}

            def og_mm(pO, tb, nb, m, ks):
                for k in ks:
                    nc.tensor.matmul(pO[:m, :],
                                     lhsT=oT_sb[:, k, tb * 128: tb * 128 + m],
                                     rhs=wo_tiles[k][:, nb * 512:(nb + 1) * 512],
                                     start=(k == 0), stop=(k == NT - 1))

            def og_out(pO, tb, nb, m):
                ob = bsm.tile([128, 512], F32, tag="ob", bufs=4,
                              name=f"ob{tb}_{nb}")
                nc.scalar.copy(out=ob[:m, :], in_=pO[:m, :])
                nc.sync.dma_start(
                    out=out.ap()[tb * 128: tb * 128 + m,
                                 nb * 512:(nb + 1) * 512],
                    in_=ob[:m, :])

            for i, (h, bi, kt) in enumerate(steps):
                if i + 2 < len(steps):
                    emit_ps(i + 2)
                # prefetch the NEXT head's kv DMAs well ahead of its first
                # logits so the boundary exp never waits on the gather
                j = i + 6
                if j < len(steps) and steps[j][1:] == (0, 0) \
                        and steps[j][0] not in st:
                    head_setup(steps[j][0])
                drain(i)
                if i == 30:
                    # wo loads issued mid-attention: off the boundary-critical
                    # DMA queue, done long before the output projection
                    for k in range(NT):
                        wt = wop.tile([128, D], BF16, tag="wo", name=f"wo{k}")
                        nc.sync.dma_start(out=wt, in_=woT.ap()[k])
                        wo_tiles.append(wt)
                if i == 290:
                    # early out-proj: heads 0..10 are finalized by now, so the
                    # first two groups' k=0..10 matmuls soak up PE slack during
                    # the last head's Act-bound steps (their PSUM slots are
                    # pos(10)'s, freed by its finalize)
                    for gi in range(2):
                        t = pb_o.tile([128, 512], F32, tag="po",
                                      name=f"pOe{gi}")
                        og_mm(t, 0, gi, 128, range(NT - 1))
                        early[gi] = t
                s = st[h]
                ps = s["ps"].pop((bi, kt))
                if bi == 4:
                    etail = bsm.tile([NTAIL, TOK], BF16, tag="etail")
                    nc.scalar.activation(out=etail, in_=ps[0:NTAIL, :],
                                         func=mybir.ActivationFunctionType.Exp,
                                         scale=SM_SCALE)
                    s["etail"] = etail
                    e_ap, lhsT_pv, i_mm = etail[:, :], s["vtail"][:, :], 4 * NKT
                else:
                    if kt == 0:
                        s["ebs"][bi] = ep.tile([128, NKT, TOK], BF16, tag="eb",
                                               name=f"eb{h}_{bi}")
                    eb = s["ebs"][bi]
                    nc.scalar.activation(out=eb[:, kt, :], in_=ps,
                                         func=mybir.ActivationFunctionType.Exp,
                                         scale=SM_SCALE)
                    e_ap, lhsT_pv = eb[:, kt, :], s["vbs"][bi][:, kt, :]
                    i_mm = bi * NKT + kt
                for qb, (q0, qn) in enumerate(QS):
                    nc.tensor.matmul(s["pos"][qb][:, :qn], lhsT=lhsT_pv,
                                     rhs=e_ap[:, q0:q0 + qn],
                                     start=(i_mm == 0), stop=(i_mm == n_mm - 1))
                if bi < 4 and kt == NKT - 1:
                    # per-block pairwise denominator tree (bf16, 2x DVE)
                    eb = s["ebs"][bi]
                    t1 = accp.tile([128, TOK], BF16, tag="dt", name="t1")
                    nc.vector.tensor_tensor(t1, eb[:, 0, :], eb[:, 1, :],
                                            mybir.AluOpType.add)
                    t2 = accp.tile([128, TOK], BF16, tag="dt", name="t2")
                    nc.vector.tensor_tensor(t2, eb[:, 2, :], eb[:, 3, :],
                                            mybir.AluOpType.add)
                    t3 = accp.tile([128, TOK], BF16, tag="dt", name="t3")
                    nc.vector.tensor_tensor(t3, eb[:, 4, :], eb[:, 5, :],
                                            mybir.AluOpType.add)
                    nc.vector.tensor_tensor(t1, t1, t2, mybir.AluOpType.add)
                    acc = accp.tile([128, TOK], BF16, tag="acc",
                                    name=f"acc{h}_{bi}")
                    nc.vector.tensor_tensor(acc, t1, t3, mybir.AluOpType.add)
                    s["accs"].append(acc)
                if bi == 4:
                    # den = (acc0+acc1+tail) + (acc2+acc3); off the PE queue
                    accs, etail = s["accs"], s["etail"]
                    s01 = accp.tile([128, TOK], BF16, tag="dt", name="s01")
                    nc.vector.tensor_tensor(s01, accs[0], accs[1],
                                            mybir.AluOpType.add)
                    nc.vector.tensor_tensor(s01[0:NTAIL, :], s01[0:NTAIL, :],
                                            etail, mybir.AluOpType.add)
                    s23 = accp.tile([128, TOK], BF16, tag="dt", name="s23")
                    nc.vector.tensor_tensor(s23, accs[2], accs[3],
                                            mybir.AluOpType.add)
                    dr = dp.tile([128, TOK], BF16, tag="dr", name=f"dr{h}")
                    nc.vector.tensor_tensor(dr, s01, s23, mybir.AluOpType.add)
                    pending.append((i + 5, "midA", h, dr, s["pos"]))
                    pending.sort(key=lambda e: e[0])
                    del st[h]
            while pending:
                _, kind, fh, fdat, fpos = pending.pop(0)
                if kind == "midA":
                    pdp = emit_midA(fh, fdat)
                    rec = emit_midB(fh, fdat, pdp)
                    emit_finalize(fh, rec, fpos)
                elif kind == "midB":
                    rec = emit_midB(fh, fdat[0], fdat[1])
                    emit_finalize(fh, rec, fpos)
                else:
                    emit_finalize(fh, fdat, fpos)

            # ---------------- out projection (attention pool scope) -----
            # groups 0-1 already hold k=0..10 from the early emission; a
            # third group runs k=0..10 before any k=11 so PE never blocks
            # on the last head's finalize writing oT[:, 11, :]
            pg2 = pb_o.tile([128, 512], F32, tag="po", name="pOi2")
            og_mm(pg2, 0, 2, 128, range(NT - 1))
            early[2] = pg2
            for gi in range(3):
                og_mm(early[gi], 0, gi, 128, [NT - 1])
                og_out(early[gi], 0, gi, 128)
            for tb, nb in groups[3:]:
                m = 128 if tb < 6 else MTAIL
                pO = pb_o.tile([128, 512], F32, tag="po",
                               name=f"pO{tb}_{nb}")
                og_mm(pO, tb, nb, m, range(NT))
                og_out(pO, tb, nb, m)

    return _finish(nc)


def _finish(nc):
    nc.compile()
    return nc


# ---------------- host-side prep ----------------

def _perm():
    p = np.arange(D).reshape(NH, C, 2)
    return np.concatenate([p[:, :, 0], p[:, :, 1]], axis=1).reshape(-1)


def make_fcis(freqs, grid_sizes):
    f, h, w = [int(v) for v in np.asarray(grid_sizes)[0]]
    c1 = C - 2 * (C // 3)
    c2 = C // 3
    fq = np.asarray(freqs, np.float32)
    ff = np.broadcast_to(fq[:f, None, None, :c1], (f, h, w, c1, 2))
    fh = np.broadcast_to(fq[None, :h, None, c1:c1 + c2], (f, h, w, c2, 2))
    fw = np.broadcast_to(fq[None, None, :w, c1 + c2:c1 + 2 * c2], (f, h, w, c2, 2))
    fcis = np.concatenate([ff, fh, fw], axis=3).reshape(f * h * w, C, 2)
    if fcis.shape[0] == 1 and S > 1:
        fcis = np.broadcast_to(fcis, (S, C, 2))
    return fcis


def host_prep(inputs):
    """inputs: the full reference input dict -> per-core in_maps."""
    import ml_dtypes
    bf16 = ml_dtypes.bfloat16
    x = np.asarray(inputs["x"], np.float32)
    freqs = np.asarray(inputs["freqs"], np.float32)
    grid_sizes = np.asarray(inputs["grid_sizes"])
    assert x.shape == (1, S, D)
    assert int(np.asarray(inputs["chunk_size"])) == S // NCH
    assert int(np.asarray(inputs["top_k"])) == 2

    perm = _perm()
    wq = np.asarray(inputs["wq"], np.float32)[perm]
    wk = np.asarray(inputs["wk"], np.float32)[perm]
    wv = np.asarray(inputs["wv"], np.float32)
    wo = np.asarray(inputs["wo"], np.float32)
    for b in ("bq", "bk", "bv", "bo"):
        assert not np.any(np.asarray(inputs[b])), f"nonzero bias {b} unsupported"
    for g in ("gq", "gk"):
        assert np.all(np.asarray(inputs[g]) == 1.0), f"non-unit gain {g} unsupported"

    xT = np.ascontiguousarray(x[0].T).reshape(NT, 128, S).astype(bf16)
    wqT = np.ascontiguousarray(wq.T).reshape(NT, 128, D).astype(bf16)
    wkT = np.ascontiguousarray(wk.T).reshape(NT, 128, D).astype(bf16)
    wvT = np.ascontiguousarray(wv.T).reshape(NT, 128, D).astype(bf16)
    woT = np.ascontiguousarray(wo.T).reshape(NT, 128, D).astype(bf16)

    fcis = make_fcis(freqs, grid_sizes)  # [S, C, 2]
    frT = fcis[:, :, 0].T  # [C, S]
    fiT = fcis[:, :, 1].T
    f11 = np.concatenate([frT, frT], axis=0)  # [128, S]
    f2n = np.concatenate([-fiT, fiT], axis=0)

    in_maps = []
    for c in range(N_CORES):
        sl = slice(c * TOK, (c + 1) * TOK)
        cm = np.zeros((128, NCH), np.float32)
        cm[:, (c * TOK) // (S // NCH)] = 1.0
        in_maps.append({
            "xT": np.ascontiguousarray(xT[:, :, sl]),
            "wqT": wqT, "wkT": wkT, "wvT": wvT, "woT": woT,
            "f11": np.ascontiguousarray(f11[:, sl]).astype(bf16),
            "f2n": np.ascontiguousarray(f2n[:, sl]).astype(bf16),
            "chmask": cm,
        })
    return in_maps


def assemble_out(results):
    return np.concatenate([r["out"] for r in results], axis=0)[None]


# ---------------- harness entry point ----------------

_CACHE = {}


def kernel(**inputs):
    import numpy as _np
    key = "nc"
    if key not in _CACHE:
        _CACHE[key] = build_kernel()
    nc = _CACHE[key]
    in_maps = host_prep(inputs)
    from concourse import bass_utils
    res = bass_utils.run_bass_kernel_spmd(
        nc, in_maps, core_ids=list(range(N_CORES)), trace=False)
    return assemble_out(res.results).astype(_np.float32)


# revision 81
# speedup vs baseline: 1.0115x; 1.0062x over previous
"""nn_CausalWanSelfAttention Trainium2 kernel (8-core SPMD, single launch).

Entry point: kernel(**inputs) -> np.ndarray [1, 6240, 1536] float32.

Strategy (v2 — bf16 datapath, packed kv, software-pipelined attention):
  - Phase A, token-sharded (780 tokens/core): q/k/v projections as bf16
    matmuls (f32 PSUM accumulation), rmsnorm via bf16 square + ones-matmul
    partition reduction. 3D-RoPE on pair-de-interleaved channels: the
    partition-half swap runs on the DMA engines (SBUF->SBUF), then
    dst = raw*[fr;fr] + swap(raw)*[-fi;fi] as same-partition bf16 ops
    (2x DVE mode); the final add fuses the phi token-reduction via
    scalar_tensor_tensor(accum_out=...). The per-token rmsnorm scale
    folds into the rope tables once per projection.
  - One AllGather ships (k^T, v) in bf16 (780 tokens per block, no pad);
    one small AllReduce combines the routing means phi_q/phi_k.
  - Top-2-of-4 chunk routing on device; per-head chunk indices drive
    dynamically-addressed DMA gathers of the selected K/V blocks.
  - Phase B, query-sharded, emitted as ONE flat software-pipelined stream
    over all (head, kv-tile) steps: the logits matmul for step i+1 is
    emitted before step i's PV so the in-order PE queue never head-blocks
    on the exp semaphore, and each head's denominator reduce / reciprocal
    / o^T normalization are deferred several steps into the next head.
    24 full 128-row kv tiles plus ONE packed tail tile assembled from the
    4 selected blocks' 12 leftover rows (25 tile-ops per engine, not 28).
    exp on the scalar engine (the pacing engine, ~87% busy); softmax
    denominator via a bf16 pairwise tree on the vector engine; PV
    accumulation in PSUM; the row-parallel output projection reads o^T
    (bf16) from SBUF.
"""

from contextlib import ExitStack

import numpy as np

import concourse.bacc as bacc
import concourse.mybir as mybir
import concourse.tile as tile

F32R = mybir.dt.float32r
F32 = mybir.dt.float32
BF16 = mybir.dt.bfloat16
KV_BYTES_PER_ELEM = 2    # K/V ship over the AllGather in bf16

N_CORES = 8
S, D, NH, HD, C = 6240, 1536, 12, 128, 64
NT = D // 128          # 12 channel tiles
TOK = S // N_CORES     # 780 tokens per core
NBLK = N_CORES         # 8 kv blocks (one per core)
NCH = 4                # routing chunks
NKT = 6                # full 128-row kv tiles per block (768 rows)
KTAIL = TOK - NKT * 128   # 12 leftover kv rows per block
NTAIL = 4 * KTAIL         # 48 packed tail rows (4 selected blocks)
EPS = 1e-6
SM_SCALE = 1.0 / float(np.sqrt(HD))
KV_ELEMS = NT * 128 * TOK  # bf16 elements per (k or v) per core
MTAIL = TOK - 6 * 128  # 12 (token tail within a core block)


def build_kernel(n_cores=N_CORES, solo=False, phases="full"):
    nc = bacc.Bacc("TRN2", target_bir_lowering=False, debug=False,
                   num_devices=n_cores)

    xT = nc.dram_tensor("xT", [NT, 128, TOK], BF16, kind="ExternalInput")
    wqT = nc.dram_tensor("wqT", [NT, 128, D], BF16, kind="ExternalInput")
    wkT = nc.dram_tensor("wkT", [NT, 128, D], BF16, kind="ExternalInput")
    wvT = nc.dram_tensor("wvT", [NT, 128, D], BF16, kind="ExternalInput")
    woT = nc.dram_tensor("woT", [NT, 128, D], BF16, kind="ExternalInput")
    f11 = nc.dram_tensor("f11", [128, TOK], BF16, kind="ExternalInput")
    f2n = nc.dram_tensor("f2n", [128, TOK], BF16, kind="ExternalInput")
    chmask = nc.dram_tensor("chmask", [128, NCH], F32, kind="ExternalInput")

    out = nc.dram_tensor("out", [TOK, D], F32, kind="ExternalOutput")

    # collective buffers
    ag_in = nc.dram_tensor("ag_in", [2, KV_ELEMS], BF16)
    ag_out = nc.dram_tensor("ag_out", [NBLK, 2, KV_ELEMS], BF16,
                            addr_space="Shared")
    phi_in = nc.dram_tensor("phi_in", [128, NT, 1 + NCH], F32)
    phi_out = nc.dram_tensor("phi_out", [128, NT, 1 + NCH], F32,
                             addr_space="Shared")
    rec_dram = nc.dram_tensor("rec_dram", [NH, TOK], BF16)

    k_in_view = ag_in.ap()[0].rearrange("(t p n) -> t p n", p=128, n=TOK)
    v_in_view = ag_in.ap()[1].rearrange("(k d) -> k d", d=D)

    ones_col_t = nc.inline_tensor(np.ones((128, 1), np.float32), name="ones_col")
    ones_row_t = nc.inline_tensor(np.ones((1, 128), np.float32), name="ones_row")

    with tile.TileContext(nc) as tc, ExitStack() as top:
        consts = top.enter_context(tc.tile_pool(name="consts", bufs=1))
        ones_col = consts.tile([128, 1], F32R)
        nc.sync.dma_start(out=ones_col, in_=ones_col_t.ap().bitcast(F32R))
        ones_row = consts.tile([1, 128], F32R)
        nc.sync.dma_start(out=ones_row, in_=ones_row_t.ap().bitcast(F32R))
        ones_col_bf = consts.tile([128, 1], BF16)
        nc.vector.memset(ones_col_bf, 1.0)
        f11_sb = consts.tile([128, TOK], BF16)
        nc.sync.dma_start(out=f11_sb, in_=f11[:, :])
        f2n_sb = consts.tile([128, TOK], BF16)
        nc.sync.dma_start(out=f2n_sb, in_=f2n[:, :])
        cm_sb = consts.tile([128, NCH], F32)
        nc.sync.dma_start(out=cm_sb, in_=chmask[:, :])
        eps_sb = consts.tile([1, 1], F32)
        nc.vector.memset(eps_sb, EPS)
        # preload the Exp activation table so the first attention exp
        # doesn't pay the 1.3us LoadActFuncSet at the phase boundary
        warm = consts.tile([1, 1], F32)
        nc.scalar.activation(out=warm, in_=eps_sb,
                             func=mybir.ActivationFunctionType.Exp)

        # persistent across phases
        persist = top.enter_context(tc.tile_pool(name="persist", bufs=1))
        qbf = persist.tile([128, NT, TOK], BF16)
        phiq_sb = persist.tile([128, NT], F32)
        phik_sb = persist.tile([128, NT], F32)

        # ---------------- Phase A ----------------
        with (
            tc.tile_pool(name="xin", bufs=1) as xin,
            tc.tile_pool(name="wts", bufs=24) as wts,
            tc.tile_pool(name="pa_mm", bufs=2, space="PSUM") as pa_mm,
            tc.tile_pool(name="pa_ss", bufs=2, space="PSUM") as pa_ss,
            tc.tile_pool(name="pa_g", bufs=1, space="PSUM") as pa_g,
            tc.tile_pool(name="raw", bufs=1) as rawp,
            tc.tile_pool(name="sqp", bufs=3) as sqp,
            tc.tile_pool(name="rope", bufs=3) as ropep,
            tc.tile_pool(name="outbf", bufs=3) as outbf,
            tc.tile_pool(name="small", bufs=2) as smallp,
            tc.tile_pool(name="frqp", bufs=2) as frqp,
        ):
            xT_sb = xin.tile([128, NT, TOK], BF16)

            def v_proj():
                # natural layout [tok, ch]; emitted between k and q proj so
                # its matmuls overlap the k rope tail on DVE. The first weight
                # block's loads interleave with the xT loads so matmul k can
                # start as soon as (xT[k], wv0[k]) land.
                for nb in range(4):
                    wv_nb = []
                    for k in range(NT):
                        if nb == 0:
                            nc.sync.dma_start(out=xT_sb[:, k, :], in_=xT.ap()[k])
                        wt = wts.tile([128, 384], BF16, tag="wt",
                                      name=f"wv{nb}_{k}")
                        nc.sync.dma_start(
                            out=wt, in_=wvT.ap()[k, :, nb * 384:(nb + 1) * 384])
                        wv_nb.append(wt)
                    for tb in range(7):
                        m = 128 if tb < 6 else MTAIL
                        pv = pa_mm.tile([128, 384], F32, tag="pmm",
                                        name=f"pv{nb}_{tb}")
                        for k in range(NT):
                            nc.tensor.matmul(
                                pv[:m, :],
                                lhsT=xT_sb[:, k, tb * 128: tb * 128 + m],
                                rhs=wv_nb[k],
                                start=(k == 0), stop=(k == NT - 1),
                            )
                        vbf = outbf.tile([128, 384], BF16, tag="vbf")
                        nc.scalar.copy(out=vbf[:m, :], in_=pv[:m, :])
                        nc.sync.dma_start(
                            out=v_in_view[tb * 128: tb * 128 + m,
                                          nb * 384:(nb + 1) * 384],
                            in_=vbf[:m, :],
                        )

            # ---- q/k projections (transposed layout [ch, tok]) ----
            QSA = [(0, 512), (512, TOK - 512)]

            def qk_proj(wdram, is_q):
                tagq = "q" if is_q else "k"
                raw = rawp.tile([128, NT, TOK], BF16, tag=f"raw{tagq}")
                psss = [pa_ss.tile([1, 512], F32, tag="pss", name=f"pss{tagq}{i}")
                        for i in range(2)]
                for half in range(4):
                    w_half = []
                    for k in range(NT):
                        wt = wts.tile([128, 384], BF16, tag="wt",
                                      name=f"w{tagq}{half}_{k}")
                        nc.sync.dma_start(
                            out=wt, in_=wdram.ap()[k, :, half * 384:(half + 1) * 384])
                        w_half.append(wt)
                    for oi in range(3):
                        ot = half * 3 + oi
                        pk = pa_mm.tile([128, TOK], F32, tag="pmm")
                        for qi, (q0, qn) in enumerate(QSA):
                            for k in range(NT):
                                nc.tensor.matmul(
                                    pk[:, q0:q0 + qn],
                                    lhsT=w_half[k][:, oi * 128:(oi + 1) * 128],
                                    rhs=xT_sb[:, k, q0:q0 + qn],
                                    start=(k == 0), stop=(k == NT - 1),
                                )
                        nc.scalar.copy(out=raw[:, ot, :], in_=pk)
                        sq = sqp.tile([128, TOK], BF16, tag="sq")
                        nc.vector.tensor_tensor(sq, raw[:, ot, :], raw[:, ot, :],
                                                mybir.AluOpType.mult)
                        for qi, (q0, qn) in enumerate(QSA):
                            nc.tensor.matmul(psss[qi][:, :qn], lhsT=ones_col_bf,
                                             rhs=sq[:, q0:q0 + qn],
                                             start=(ot == 0), stop=(ot == NT - 1))
                rs = smallp.tile([1, TOK], F32R, tag="rs")
                for qi, (q0, qn) in enumerate(QSA):
                    rs1 = smallp.tile([1, 512], F32, tag="rs1")
                    nc.scalar.activation(out=rs1[:, :qn], in_=psss[qi][:, :qn],
                                         func=mybir.ActivationFunctionType.Sqrt,
                                         bias=eps_sb[0:1, 0:1], scale=1.0 / D)
                    with nc.allow_low_precision(reason="rms scale in f32r"):
                        nc.vector.reciprocal(out=rs[:, q0:q0 + qn], in_=rs1[:, :qn])
                # broadcast rs to 128 partitions, fold into the rope tables
                prs = pa_g.tile([128, TOK], F32, tag="pg", name=f"prs{tagq}")
                for qi, (q0, qn) in enumerate(QSA):
                    nc.tensor.matmul(prs[:, q0:q0 + qn], lhsT=ones_row,
                                     rhs=rs[:, q0:q0 + qn], start=True, stop=True)
                prs_bf = smallp.tile([128, TOK], BF16, tag="prsbf")
                nc.scalar.copy(out=prs_bf, in_=prs)
                G1 = frqp.tile([128, TOK], BF16, tag="G1")
                nc.vector.tensor_tensor(G1, f11_sb, prs_bf, mybir.AluOpType.mult)
                G2 = frqp.tile([128, TOK], BF16, tag="G2")
                nc.vector.tensor_tensor(G2, f2n_sb, prs_bf, mybir.AluOpType.mult)
                for ot in range(NT):
                    # rope: pairs de-interleaved -> a=rows 0:C, b=rows C:128;
                    # dst = raw*[fr;fr] + swap(raw)*[-fi;fi], swap via DMA
                    rsw = ropep.tile([128, TOK], BF16, tag="rsw", bufs=12)
                    nc.sync.dma_start(out=rsw[0:C, :], in_=raw[C:128, ot, :])
                    nc.sync.dma_start(out=rsw[C:128, :], in_=raw[0:C, ot, :])
                    m1 = ropep.tile([128, TOK], BF16, tag="m1", bufs=6)
                    nc.vector.tensor_tensor(m1, raw[:, ot, :], G1,
                                            mybir.AluOpType.mult)
                    m2 = ropep.tile([128, TOK], BF16, tag="m2", bufs=6)
                    nc.vector.tensor_tensor(m2, rsw, G2,
                                            mybir.AluOpType.mult)
                    if is_q:
                        dst = qbf[:, ot, :]
                    else:
                        kbf = outbf.tile([128, TOK], BF16, tag="kbf")
                        dst = kbf[:, :]
                    phi_dst = phiq_sb if is_q else phik_sb
                    nc.vector.scalar_tensor_tensor(
                        out=dst, in0=m1, scalar=1.0, in1=m2,
                        op0=mybir.AluOpType.mult, op1=mybir.AluOpType.add,
                        accum_out=phi_dst[:, ot: ot + 1])
                    if not is_q:
                        nc.sync.dma_start(out=k_in_view[ot, :, 0:TOK], in_=kbf)

            v_proj()
            qk_proj(wkT, is_q=False)

            # AllGather (kT, v) once k and v blocks are written
            if not solo:
                nc.gpsimd.collective_compute(
                    "AllGather", mybir.AluOpType.bypass,
                    replica_groups=[list(range(n_cores))],
                    ins=[ag_in.ap().opt()], outs=[ag_out.ap().opt()],
                )

            qk_proj(wqT, is_q=True)

            # ---- phi AllReduce ----
            nc.sync.dma_start(out=phi_in.ap()[:, :, 0:1],
                              in_=phiq_sb[:, :, None])
            phik_m = smallp.tile([128, NT, NCH], F32, tag="phikm")
            for ch in range(NCH):
                nc.vector.tensor_scalar_mul(phik_m[:, :, ch], phik_sb,
                                            cm_sb[:, ch: ch + 1])
            nc.sync.dma_start(out=phi_in.ap()[:, :, 1: 1 + NCH], in_=phik_m)
            if not solo:
                nc.gpsimd.collective_compute(
                    "AllReduce", mybir.AluOpType.add,
                    replica_groups=[list(range(n_cores))],
                    ins=[phi_in.ap().opt()], outs=[phi_out.ap().opt()],
                )

            # ---- routing scores + top-2 chunk indices ----
            phis = smallp.tile([128, NT, 1 + NCH], F32, tag="phis")
            nc.sync.dma_start(out=phis, in_=(phi_in if solo else phi_out).ap())
            prod = smallp.tile([128, NT, NCH], F32R, tag="prodsc")
            nc.vector.tensor_tensor(
                prod, phis[:, :, 1: 1 + NCH],
                phis[:, :, 0:1].to_broadcast((128, NT, NCH)),
                mybir.AluOpType.mult)
            psc = pa_ss.tile([1, NH * NCH], F32, tag="pss", name="psc")
            nc.tensor.matmul(psc, lhsT=ones_col,
                             rhs=prod[:, :, :].rearrange("p t c -> p (t c)"),
                             start=True, stop=True)
            sc = smallp.tile([1, NH * NCH], F32, tag="sc")
            nc.vector.tensor_copy(out=sc, in_=psc)
            scv = sc[:, :].rearrange("p (h c) -> p h c", c=NCH)
            m1s = smallp.tile([1, NH], F32, tag="m1s")
            nc.vector.reduce_max(out=m1s, in_=scv, axis=mybir.AxisListType.X)
            is1 = smallp.tile([1, NH * NCH], F32, tag="is1")
            nc.vector.tensor_tensor(
                is1[:, :].rearrange("p (h c) -> p h c", c=NCH),
                scv, m1s[:, :, None].to_broadcast((1, NH, NCH)),
                mybir.AluOpType.is_ge)
            nc.vector.tensor_scalar_mul(is1, is1, 1e30)
            masked = smallp.tile([1, NH * NCH], F32, tag="masked")
            nc.vector.tensor_tensor(masked, sc, is1, mybir.AluOpType.subtract)
            m2s = smallp.tile([1, NH], F32, tag="m2s")
            nc.vector.reduce_max(out=m2s,
                                 in_=masked[:, :].rearrange("p (h c) -> p h c", c=NCH),
                                 axis=mybir.AxisListType.X)
            # chunk indices: i1 = argmax, i2 = arg-2nd-max (per head)
            iota4 = smallp.tile([1, NCH], F32, tag="iota4")
            nc.gpsimd.iota(iota4.bitcast(mybir.dt.int32), pattern=[[1, NCH]],
                           base=0, channel_multiplier=0)
            nc.vector.tensor_copy(out=iota4, in_=iota4.bitcast(mybir.dt.int32))
            is2 = smallp.tile([1, NH * NCH], F32, tag="is2")
            nc.vector.tensor_tensor(
                is2[:, :].rearrange("p (h c) -> p h c", c=NCH),
                masked[:, :].rearrange("p (h c) -> p h c", c=NCH),
                m2s[:, :, None].to_broadcast((1, NH, NCH)),
                mybir.AluOpType.is_ge)
            nc.vector.tensor_scalar_mul(is1, is1, 1e-30)  # undo 1e30 scale -> 0/1
            idxf = smallp.tile([1, NH, 2], F32, tag="idxf")
            w1 = smallp.tile([1, NH * NCH], F32, tag="w1")
            nc.vector.tensor_tensor(
                w1[:, :].rearrange("p (h c) -> p h c", c=NCH),
                is1[:, :].rearrange("p (h c) -> p h c", c=NCH),
                iota4[:, None, :].to_broadcast((1, NH, NCH)),
                mybir.AluOpType.mult)
            nc.vector.reduce_sum(out=idxf[:, :, 0], in_=w1[:, :].rearrange(
                "p (h c) -> p h c", c=NCH), axis=mybir.AxisListType.X)
            nc.vector.tensor_tensor(
                w1[:, :].rearrange("p (h c) -> p h c", c=NCH),
                is2[:, :].rearrange("p (h c) -> p h c", c=NCH),
                iota4[:, None, :].to_broadcast((1, NH, NCH)),
                mybir.AluOpType.mult)
            nc.vector.reduce_sum(out=idxf[:, :, 1], in_=w1[:, :].rearrange(
                "p (h c) -> p h c", c=NCH), axis=mybir.AxisListType.X)
            idx_i32 = persist.tile([1, NH * 2], mybir.dt.int32)
            nc.vector.tensor_copy(out=idx_i32,
                                  in_=idxf[:, :, :].rearrange("p h s -> p (h s)"))

        # ---------------- Phase B: attention ----------------
        otp = top.enter_context(tc.tile_pool(name="otp", bufs=1))
        oT_sb = otp.tile([128, NT, TOK], BF16)
        if phases == "a":
            return _finish(nc)
        QS = [(0, 512), (512, TOK - 512)]  # bank-aligned query splits
        wop = top.enter_context(tc.tile_pool(name="wo", bufs=12))
        from concourse.bass import ds as _ds
        with (
            tc.tile_pool(name="kv", bufs=10) as kvp,
            tc.tile_pool(name="ktl", bufs=2) as ktlp,
            tc.tile_pool(name="ebf", bufs=3) as ep,
            tc.tile_pool(name="accp", bufs=6) as accp,
            tc.tile_pool(name="dacc", bufs=2) as dp,
            tc.tile_pool(name="bsm", bufs=2) as bsm,
            tc.tile_pool(name="pb_s", bufs=2, space="PSUM") as pb_s,
            tc.tile_pool(name="pb_o", bufs=4, space="PSUM") as pb_o,
        ):
            # flat software-pipelined stream over all (head, kv-tile) steps:
            # the logits matmul for step i+1 is emitted BEFORE step i's PV so
            # the in-order PE queue never head-blocks on the exp semaphore.
            n_mm = 4 * NKT + 1
            steps = [(h, bi, kt) for h in range(NH)
                     for bi, kt in ([(b, k) for b in range(4)
                                     for k in range(NKT)] + [(4, 0)])]
            st = {}

            def head_setup(h):
                pos = [pb_o.tile([128, 512], F32, tag="po", name=f"po{h}_{qb}")
                       for qb in range(2)]
                blk_regs = []
                for sel in range(2):
                    iv = nc.values_load(
                        idx_i32[0:1, h * 2 + sel: h * 2 + sel + 1],
                        min_val=0, max_val=NCH - 1,
                        skip_runtime_bounds_check=True)
                    blk_regs.append(iv)
                ktail = ktlp.tile([128, NTAIL], BF16, tag="ktail")
                vtail = ktlp.tile([NTAIL, HD], BF16, tag="vtail")
                kts, vbs = [], []
                for bi in range(4):
                    blk = blk_regs[bi // 2] * 2 + (bi % 2)
                    kv_b = (ag_in.ap() if solo
                            else ag_out.ap()[_ds(blk, 1)][0])
                    kT_b = kvp.tile([128, TOK], BF16, tag="kb",
                                    name=f"kT{h}_{bi}")
                    k_view = kv_b[0].rearrange("(t p n) -> t p n",
                                               p=128, n=TOK)[h]
                    nc.sync.dma_start(out=kT_b, in_=k_view)
                    nc.sync.dma_start(
                        out=ktail[:, bi * KTAIL:(bi + 1) * KTAIL],
                        in_=k_view[:, NKT * 128: TOK])
                    V_b = kvp.tile([128, NKT, HD], BF16, tag="vb",
                                   name=f"V{h}_{bi}")
                    nc.sync.dma_start(
                        out=V_b,
                        in_=kv_b[1][0:NKT * 128 * D]
                        .rearrange("(n p d) -> p n d", p=128, d=D)
                        [:, :, h * HD:(h + 1) * HD])
                    nc.sync.dma_start(
                        out=vtail[bi * KTAIL:(bi + 1) * KTAIL, :],
                        in_=kv_b[1].rearrange("(t d) -> t d", d=D)
                        [NKT * 128: TOK, h * HD:(h + 1) * HD])
                    kts.append(kT_b)
                    vbs.append(V_b)
                st[h] = dict(pos=pos, ktail=ktail, vtail=vtail, kts=kts,
                             vbs=vbs, ebs={}, accs=[], ps={}, etail=None)

            def emit_ps(i):
                h, bi, kt = steps[i]
                if (bi, kt) == (0, 0) and h not in st:
                    head_setup(h)
                s = st[h]
                ps = pb_s.tile([128, TOK], F32, tag="ps", name=f"ps{i}")
                if bi == 4:
                    lhs, rows = s["ktail"][:, :], NTAIL
                else:
                    lhs, rows = s["kts"][bi][:, kt * 128:(kt + 1) * 128], 128
                for qb, (q0, qn) in enumerate(QS):
                    nc.tensor.matmul(ps[0:rows, q0:q0 + qn], lhsT=lhs,
                                     rhs=qbf[:, h, q0:q0 + qn],
                                     start=True, stop=True)
                s["ps"][(bi, kt)] = ps

            pending = []

            def emit_midA(h, dr):
                # denominator partition-reduce, first query half only: the
                # two pdp matmuls are split across two drain slots so each
                # displaces the logits stream by half as much
                pdp = pb_s.tile([1, TOK], F32, tag="ps", name=f"pdp{h}")
                q0, qn = QS[0]
                nc.tensor.matmul(pdp[:, q0:q0 + qn], lhsT=ones_col_bf,
                                 rhs=dr[:, q0:q0 + qn], start=True, stop=True)
                return pdp

            def emit_midB(h, dr, pdp):
                q0, qn = QS[1]
                nc.tensor.matmul(pdp[:, q0:q0 + qn], lhsT=ones_col_bf,
                                 rhs=dr[:, q0:q0 + qn], start=True, stop=True)
                rec = bsm.tile([1, TOK], BF16, tag="rec", name=f"rec{h}")
                with nc.allow_low_precision(reason="softmax denom"):
                    nc.vector.reciprocal(out=rec, in_=pdp)
                # DRAM round-trip broadcast: replicate the reciprocal row to
                # 128 partitions on the DMA engines (keeps PE out of the
                # per-head tail entirely)
                nc.sync.dma_start(out=rec_dram.ap()[h:h + 1, :], in_=rec)
                return rec

            def emit_finalize(h, rec, pos):
                rb_sb = bsm.tile([128, TOK], BF16, tag="rbsb")
                nc.sync.dma_start(
                    out=rb_sb,
                    in_=rec_dram.ap()[h:h + 1, :].to_broadcast((128, TOK)))
                for qb, (q0, qn) in enumerate(QS):
                    po_bf = bsm.tile([128, 512], BF16, tag="pobf",
                                     name=f"pobf{h}_{qb}")
                    nc.vector.tensor_copy(out=po_bf[:, :qn],
                                          in_=pos[qb][:, :qn])
                    nc.vector.tensor_tensor(oT_sb[:, h, q0:q0 + qn],
                                            po_bf[:, :qn],
                                            rb_sb[:, q0:q0 + qn],
                                            mybir.AluOpType.mult)

            def drain(i):
                while pending and pending[0][0] <= i:
                    _, kind, fh, fdat, fpos = pending.pop(0)
                    if kind == "midA":
                        pdp = emit_midA(fh, fdat)
                        pending.append((i + 1, "midB", fh, (fdat, pdp), fpos))
                        pending.sort(key=lambda e: e[0])
                    elif kind == "midB":
                        rec = emit_midB(fh, fdat[0], fdat[1])
                        pending.append((i + 3, "fin", fh, rec, fpos))
                        pending.sort(key=lambda e: e[0])
                    else:
                        emit_finalize(fh, fdat, fpos)

            emit_ps(0)
            emit_ps(1)
            wo_tiles = []
            groups = [(tb, nb) for tb in range(7) for nb in range(3)]
            early = {}

            def og_mm(pO, tb, nb, m, ks):
                for k in ks:
                    nc.tensor.matmul(pO[:m, :],
                                     lhsT=oT_sb[:, k, tb * 128: tb * 128 + m],
                                     rhs=wo_tiles[k][:, nb * 512:(nb + 1) * 512],
                                     start=(k == 0), stop=(k == NT - 1))

            def og_out(pO, tb, nb, m):
                ob = bsm.tile([128, 512], F32, tag="ob", bufs=4,
                              name=f"ob{tb}_{nb}")
                nc.scalar.copy(out=ob[:m, :], in_=pO[:m, :])
                nc.sync.dma_start(
                    out=out.ap()[tb * 128: tb * 128 + m,
                                 nb * 512:(nb + 1) * 512],
                    in_=ob[:m, :])

            for i, (h, bi, kt) in enumerate(steps):
                if i + 2 < len(steps):
                    emit_ps(i + 2)
                # prefetch the NEXT head's kv DMAs well ahead of its first
                # logits so the boundary exp never waits on the gather
                j = i + 6
                if j < len(steps) and steps[j][1:] == (0, 0) \
                        and steps[j][0] not in st:
                    head_setup(steps[j][0])
                drain(i)
                if i == 30:
                    # wo loads issued mid-attention: off the boundary-critical
                    # DMA queue, done long before the output projection
                    for k in range(NT):
                        wt = wop.tile([128, D], BF16, tag="wo", name=f"wo{k}")
                        nc.sync.dma_start(out=wt, in_=woT.ap()[k])
                        wo_tiles.append(wt)
                if i == 290:
                    # early out-proj: heads 0..10 are finalized by now, so the
                    # first two groups' k=0..10 matmuls soak up PE slack during
                    # the last head's Act-bound steps (their PSUM slots are
                    # pos(10)'s, freed by its finalize)
                    for gi in range(2):
                        t = pb_o.tile([128, 512], F32, tag="po",
                                      name=f"pOe{gi}")
                        og_mm(t, 0, gi, 128, range(NT - 1))
                        early[gi] = t
                s = st[h]
                ps = s["ps"].pop((bi, kt))
                if bi == 4:
                    etail = bsm.tile([NTAIL, TOK], BF16, tag="etail")
                    nc.scalar.activation(out=etail, in_=ps[0:NTAIL, :],
                                         func=mybir.ActivationFunctionType.Exp,
                                         scale=SM_SCALE)
                    s["etail"] = etail
                    e_ap, lhsT_pv, i_mm = etail[:, :], s["vtail"][:, :], 4 * NKT
                else:
                    if kt == 0:
                        s["ebs"][bi] = ep.tile([128, NKT, TOK], BF16, tag="eb",
                                               name=f"eb{h}_{bi}")
                    eb = s["ebs"][bi]
                    nc.scalar.activation(out=eb[:, kt, :], in_=ps,
                                         func=mybir.ActivationFunctionType.Exp,
                                         scale=SM_SCALE)
                    e_ap, lhsT_pv = eb[:, kt, :], s["vbs"][bi][:, kt, :]
                    i_mm = bi * NKT + kt
                for qb, (q0, qn) in enumerate(QS):
                    nc.tensor.matmul(s["pos"][qb][:, :qn], lhsT=lhsT_pv,
                                     rhs=e_ap[:, q0:q0 + qn],
                                     start=(i_mm == 0), stop=(i_mm == n_mm - 1))
                if bi < 4 and kt == NKT - 1:
                    # per-block pairwise denominator tree (bf16, 2x DVE)
                    eb = s["ebs"][bi]
                    t1 = accp.tile([128, TOK], BF16, tag="dt", name="t1")
                    nc.vector.tensor_tensor(t1, eb[:, 0, :], eb[:, 1, :],
                                            mybir.AluOpType.add)
                    t2 = accp.tile([128, TOK], BF16, tag="dt", name="t2")
                    nc.vector.tensor_tensor(t2, eb[:, 2, :], eb[:, 3, :],
                                            mybir.AluOpType.add)
                    t3 = accp.tile([128, TOK], BF16, tag="dt", name="t3")
                    nc.vector.tensor_tensor(t3, eb[:, 4, :], eb[:, 5, :],
                                            mybir.AluOpType.add)
                    nc.vector.tensor_tensor(t1, t1, t2, mybir.AluOpType.add)
                    acc = accp.tile([128, TOK], BF16, tag="acc",
                                    name=f"acc{h}_{bi}")
                    nc.vector.tensor_tensor(acc, t1, t3, mybir.AluOpType.add)
                    s["accs"].append(acc)
                if bi == 4:
                    # den = (acc0+acc1+tail) + (acc2+acc3); off the PE queue
                    accs, etail = s["accs"], s["etail"]
                    s01 = accp.tile([128, TOK], BF16, tag="dt", name="s01")
                    nc.vector.tensor_tensor(s01, accs[0], accs[1],
                                            mybir.AluOpType.add)
                    nc.vector.tensor_tensor(s01[0:NTAIL, :], s01[0:NTAIL, :],
                                            etail, mybir.AluOpType.add)
                    s23 = accp.tile([128, TOK], BF16, tag="dt", name="s23")
                    nc.vector.tensor_tensor(s23, accs[2], accs[3],
                                            mybir.AluOpType.add)
                    dr = dp.tile([128, TOK], BF16, tag="dr", name=f"dr{h}")
                    nc.vector.tensor_tensor(dr, s01, s23, mybir.AluOpType.add)
                    pending.append((i + 5, "midA", h, dr, s["pos"]))
                    pending.sort(key=lambda e: e[0])
                    del st[h]
            while pending:
                _, kind, fh, fdat, fpos = pending.pop(0)
                if kind == "midA":
                    pdp = emit_midA(fh, fdat)
                    rec = emit_midB(fh, fdat, pdp)
                    emit_finalize(fh, rec, fpos)
                elif kind == "midB":
                    rec = emit_midB(fh, fdat[0], fdat[1])
                    emit_finalize(fh, rec, fpos)
                else:
                    emit_finalize(fh, fdat, fpos)

            # ---------------- out projection (attention pool scope) -----
            # groups 0-1 already hold k=0..10 from the early emission; a
            # third group runs k=0..10 before any k=11 so PE never blocks
            # on the last head's finalize writing oT[:, 11, :]
            pg2 = pb_o.tile([128, 512], F32, tag="po", name="pOi2")
            og_mm(pg2, 0, 2, 128, range(NT - 1))
            early[2] = pg2
            for gi in range(3):
                og_mm(early[gi], 0, gi, 128, [NT - 1])
                og_out(early[gi], 0, gi, 128)
            for tb, nb in groups[3:]:
                m = 128 if tb < 6 else MTAIL
                pO = pb_o.tile([128, 512], F32, tag="po",
                               name=f"pO{tb}_{nb}")
                og_mm(pO, tb, nb, m, range(NT))
                og_out(pO, tb, nb, m)

    return _finish(nc)


def _finish(nc):
    nc.compile()
    return nc


# ---------------- host-side prep ----------------

def _perm():
    p = np.arange(D).reshape(NH, C, 2)
    return np.concatenate([p[:, :, 0], p[:, :, 1]], axis=1).reshape(-1)


def make_fcis(freqs, grid_sizes):
    f, h, w = [int(v) for v in np.asarray(grid_sizes)[0]]
    c1 = C - 2 * (C // 3)
    c2 = C // 3
    fq = np.asarray(freqs, np.float32)
    ff = np.broadcast_to(fq[:f, None, None, :c1], (f, h, w, c1, 2))
    fh = np.broadcast_to(fq[None, :h, None, c1:c1 + c2], (f, h, w, c2, 2))
    fw = np.broadcast_to(fq[None, None, :w, c1 + c2:c1 + 2 * c2], (f, h, w, c2, 2))
    fcis = np.concatenate([ff, fh, fw], axis=3).reshape(f * h * w, C, 2)
    if fcis.shape[0] == 1 and S > 1:
        fcis = np.broadcast_to(fcis, (S, C, 2))
    return fcis


def host_prep(inputs):
    """inputs: the full reference input dict -> per-core in_maps."""
    import ml_dtypes
    bf16 = ml_dtypes.bfloat16
    x = np.asarray(inputs["x"], np.float32)
    freqs = np.asarray(inputs["freqs"], np.float32)
    grid_sizes = np.asarray(inputs["grid_sizes"])
    assert x.shape == (1, S, D)
    assert int(np.asarray(inputs["chunk_size"])) == S // NCH
    assert int(np.asarray(inputs["top_k"])) == 2

    perm = _perm()
    wq = np.asarray(inputs["wq"], np.float32)[perm]
    wk = np.asarray(inputs["wk"], np.float32)[perm]
    wv = np.asarray(inputs["wv"], np.float32)
    wo = np.asarray(inputs["wo"], np.float32)
    for b in ("bq", "bk", "bv", "bo"):
        assert not np.any(np.asarray(inputs[b])), f"nonzero bias {b} unsupported"
    for g in ("gq", "gk"):
        assert np.all(np.asarray(inputs[g]) == 1.0), f"non-unit gain {g} unsupported"

    xT = np.ascontiguousarray(x[0].T).reshape(NT, 128, S).astype(bf16)
    wqT = np.ascontiguousarray(wq.T).reshape(NT, 128, D).astype(bf16)
    wkT = np.ascontiguousarray(wk.T).reshape(NT, 128, D).astype(bf16)
    wvT = np.ascontiguousarray(wv.T).reshape(NT, 128, D).astype(bf16)
    woT = np.ascontiguousarray(wo.T).reshape(NT, 128, D).astype(bf16)

    fcis = make_fcis(freqs, grid_sizes)  # [S, C, 2]
    frT = fcis[:, :, 0].T  # [C, S]
    fiT = fcis[:, :, 1].T
    f11 = np.concatenate([frT, frT], axis=0)  # [128, S]
    f2n = np.concatenate([-fiT, fiT], axis=0)

    in_maps = []
    for c in range(N_CORES):
        sl = slice(c * TOK, (c + 1) * TOK)
        cm = np.zeros((128, NCH), np.float32)
        cm[:, (c * TOK) // (S // NCH)] = 1.0
        in_maps.append({
            "xT": np.ascontiguousarray(xT[:, :, sl]),
            "wqT": wqT, "wkT": wkT, "wvT": wvT, "woT": woT,
            "f11": np.ascontiguousarray(f11[:, sl]).astype(bf16),
            "f2n": np.ascontiguousarray(f2n[:, sl]).astype(bf16),
            "chmask": cm,
        })
    return in_maps


def assemble_out(results):
    return np.concatenate([r["out"] for r in results], axis=0)[None]


# ---------------- harness entry point ----------------

_CACHE = {}


def kernel(**inputs):
    import numpy as _np
    key = "nc"
    if key not in _CACHE:
        _CACHE[key] = build_kernel()
    nc = _CACHE[key]
    in_maps = host_prep(inputs)
    from concourse import bass_utils
    res = bass_utils.run_bass_kernel_spmd(
        nc, in_maps, core_ids=list(range(N_CORES)), trace=False)
    return assemble_out(res.results).astype(_np.float32)
